# revision 3
# baseline (speedup 1.0000x reference)
"""Trainium2 Bass kernel for nn_BaselineAttnDecoder — feature-major,
software-pipelined.

Per core: 160 decode rows (16 images x 10 rounds), weights replicated.

- GRU gates / out-proj / icT / qcT all FEATURE-major: out [feat<=128, 160]
  PSUM groups, moving dim = true batch 160 (no 128+32 padding waste).
- h lives only as per-tile hT (bf16) + hF (f32): no h transposes.
- qcT via symmetric diag-trick (lhsT = batch-major q_value, rhs = the
  qw diagonal) — attention-weighted sum lands directly feature-major.
- Attention score chain (a, qe, iw softmax, diag build) for step t+1 is
  emitted in step t's tail so the DVE chain overlaps PE matmul work.
- Gate elementwise math runs per h-tile, pipelined across Act/DVE/Pool.
- Step-19 vocab argmax: blockwise top-8 from PSUM chunks (bf16 copies on
  Act, Max/MaxIndex on DVE), exact-f32 rescore of 8 candidates.
"""
import numpy as np

import concourse.bass as bass
import concourse.bacc as bacc
import concourse.mybir as mybir
import concourse.tile as tile
from concourse.masks import make_identity

F32 = mybir.dt.float32
BF16 = mybir.dt.float16  # 16-bit compute dtype (f16: 10-bit mantissa)
U32 = mybir.dt.uint32
FP8 = mybir.dt.float8e4
PM = mybir.MatmulPerfMode
AF = mybir.ActivationFunctionType
ALU = mybir.AluOpType
AX = mybir.AxisListType

D, H, V, K = 300, 512, 8835, 50
L, MAX_LEN, ROUNDS = 20, 21, 10
BS = 160
NCORES = 8
PBS = [128, 32]
BOFF = [0, 128]
VP = 18 * 512
NEG = -60000.0


def bcast_mid(ap, reps):
    return bass.AP(tensor=ap.tensor, offset=ap.offset,
                   ap=[ap.ap[0], [0, reps], ap.ap[1]])


def bcast_in(ap, reps):
    return bass.AP(tensor=ap.tensor, offset=ap.offset,
                   ap=[ap.ap[0], ap.ap[1], [0, reps]])


def build_nc():
    nc = bacc.Bacc()

    def din(name, shape, dt):
        return nc.dram_tensor(name, shape, dt, kind="ExternalInput")

    w_gi = din("w_gi", [128, 11, 3 * H], BF16)
    w_gh = din("w_gh", [128, 4, 3 * H], BF16)
    bhh_n = din("bhh_n", [1, H], BF16)
    w_egi = din("w_egi", [128, 3, 3 * H], BF16)
    w_egh = din("w_egh", [128, 4, 3 * H], BF16)
    ebhh_n = din("ebhh_n", [1, H], BF16)
    w_out = din("w_out", [128, 12, D], BF16)
    outb = din("outb", [1, D], BF16)
    w_qk = din("w_qk", [128, 4, K], BF16)
    w_qv = din("w_qv", [128, 4, H], BF16)
    w_ak = din("w_ak", [128, 4, K], BF16)
    akb = din("akb", [1, K], BF16)
    w_ik = din("w_ik", [128, 2, K], BF16)
    w_iv = din("w_iv", [128, 2, H], BF16)
    ivb_c = din("ivb_c", [128, 4], F32)
    qvb_c = din("qvb_c", [128, 4], F32)
    img_t = din("img_t", [128, 2, 2 * 128], BF16)
    emb_bf = din("emb_bf", [V, D], BF16)
    emb_aug = din("emb_aug", [V, D + 1], F32)
    embt_bf = din("embt_bf", [128, 3, VP], FP8)
    q_idx = din("q_idx", [128, 2 * L], U32)
    a_idx = din("a_idx", [128, 2 * L], U32)
    qe_mask = din("qe_mask", [128, 2, L], BF16)
    ie_mask = din("ie_mask", [128, 2, 2 * 128], BF16)

    out_o = nc.dram_tensor("out_o", [MAX_LEN, 128, 3, BS], F32,
                           kind="ExternalOutput")
    out_o19 = nc.dram_tensor("out_o19", [128, 2, D], F32,
                             kind="ExternalOutput")

    with tile.TileContext(nc) as tc:
        with (
            tc.tile_pool(name="cw", bufs=1) as cw,
            tc.tile_pool(name="pers", bufs=1) as pers,
            tc.tile_pool(name="wk", bufs=2) as wk,
            tc.tile_pool(name="st", bufs=2) as st,
            tc.tile_pool(name="ps", bufs=5, space="PSUM") as psp,
        ):
            def load(pool, t, dt):
                s = pool.tile(list(t.shape), dt, name=t.name + "_sb")
                nc.sync.dma_start(s[:], t[:])
                return s

            s_qk = load(cw, w_qk, BF16)
            s_qv = load(cw, w_qv, BF16)
            s_ak = load(cw, w_ak, BF16)
            s_ik = load(cw, w_ik, BF16)
            s_iv = load(cw, w_iv, BF16)
            s_imgt = load(cw, img_t, BF16)
            s_bhh = load(cw, bhh_n, BF16)
            s_ebhh = load(cw, ebhh_n, BF16)
            s_outb = load(cw, outb, BF16)
            s_akb = load(cw, akb, BF16)
            s_ivb = load(cw, ivb_c, F32)
            s_qvb = load(cw, qvb_c, F32)
            s_qidx = load(cw, q_idx, U32)
            s_aidx = load(cw, a_idx, U32)
            s_qem = load(cw, qe_mask, BF16)
            s_iem = load(cw, ie_mask, BF16)

            ident_bf = cw.tile([128, 128], BF16)
            make_identity(nc, ident_bf[:])
            ones_bf = cw.tile([1, BS], BF16)
            nc.vector.memset(ones_bf[:], 1.0)
            sid4 = cw.tile([128, 32], BF16)
            for g4 in range(4):
                nc.vector.tensor_copy(sid4[32 * g4:32 * (g4 + 1), :],
                                      ident_bf[0:32, 0:32])
            iota8 = cw.tile([128, 8], F32)
            nc.gpsimd.iota(iota8[:], pattern=[[1, 8]], base=0,
                           channel_multiplier=0,
                           allow_small_or_imprecise_dtypes=True)
            iota144 = cw.tile([128, 144], F32)
            nc.gpsimd.iota(iota144[:], pattern=[[1, 144]], base=0,
                           channel_multiplier=0,
                           allow_small_or_imprecise_dtypes=True)
            off18 = cw.tile([128, 144], F32)
            nc.gpsimd.iota(off18[:], pattern=[[512, 18], [0, 8]], base=0,
                           channel_multiplier=0,
                           allow_small_or_imprecise_dtypes=True)
            # identity replicated along an inner L/5 axis (for diag builds
            # that keep innermost stride-1 so DVE 2x mode applies)
            i_rep = cw.tile([128, 128, L], BF16)
            for l in range(L):
                nc.vector.tensor_copy(i_rep[:, :, l:l + 1],
                                      bass.AP(tensor=ident_bf.tensor,
                                              offset=ident_bf[:, :].offset,
                                              ap=[ident_bf[:, :].ap[0],
                                                  [1, 128], [0, 1]]))
            ident_f8 = cw.tile([128, 128], FP8)
            nc.vector.tensor_copy(ident_f8[:], ident_bf[:])
            sid_rep = cw.tile([128, 32, 5], BF16)
            for c in range(5):
                nc.vector.tensor_copy(sid_rep[:, :, c:c + 1],
                                      bass.AP(tensor=sid4.tensor,
                                              offset=sid4[:, :].offset,
                                              ap=[sid4[:, :].ap[0],
                                                  [1, 32], [0, 1]]))

            # persistent state — h per tile, feature-major
            hTs = [pers.tile([128, BS], BF16, name=f"hT{i}") for i in range(4)]
            hFs = [pers.tile([128, BS], F32, name=f"hF{i}") for i in range(4)]
            qk_b0 = pers.tile([128, L, K], BF16)
            qk_b1 = pers.tile([128, L, K], BF16)
            qkbs = [qk_b0, qk_b1]
            qv_b0 = pers.tile([128, L, H], BF16)
            qv_p1 = pers.tile([128, 5, H], BF16)
            ivv = pers.tile([128, 2, H], BF16)
            ikt2 = pers.tile([128, 2, 128], BF16)
            dec20 = pers.tile([128, 3, BS], BF16)

            for i in range(4):
                nc.vector.memset(hTs[i][:], 0.0)
                nc.vector.memset(hFs[i][:], 0.0)
            nc.vector.memset(dec20[32:64, 2, :], 0.0)
            nc.vector.memset(dec20[64:65, 2, :], 1.0)

            def tr(dst_sb_ap, src_sb_ap, pb, w, eng=None):
                p = psp.tile([128, 128], BF16, tag="trp", bufs=2, name="pt")
                nc.tensor.transpose(p[:w, :pb], src_sb_ap, ident_bf[:pb, :pb])
                (eng or nc.vector).tensor_copy(dst_sb_ap, p[:w, :pb])

            def fetch_x(idx_sb, t):
                xt = wk.tile([128, 3, BS], BF16, tag="xt", bufs=3, name="xt")
                nc.vector.memset(xt[32:64, 2, :], 0.0)
                nc.vector.memset(xt[64:65, 2, :], 1.0)
                for c, (pb, off) in enumerate(zip(PBS, BOFF)):
                    g = wk.tile([128, D], BF16, tag="gath", bufs=6, name="g")
                    nc.gpsimd.indirect_dma_start(
                        out=g[:pb], out_offset=None, in_=emb_bf[:],
                        in_offset=bass.IndirectOffsetOnAxis(
                            ap=idx_sb[:pb, 2 * t + c:2 * t + c + 1], axis=0))
                    for kt in range(3):
                        w = 128 if kt < 2 else D - 256
                        if kt == 1:
                            p = psp.tile([128, 128], BF16, tag="trp", bufs=2,
                                         name="pt")
                            nc.tensor.transpose(p[:w, :pb],
                                                g[:pb, 128:128 + w],
                                                ident_bf[:pb, :pb])
                            nc.scalar.copy(xt[:w, kt, off:off + pb],
                                           p[:w, :pb])
                        else:
                            tr(xt[:w, kt, off:off + pb],
                               g[:pb, kt * 128:kt * 128 + w], pb, w)
                return xt

            def emit_group(ps_ap, pairs):
                n = len(pairs)
                for i, (lh, rh) in enumerate(pairs):
                    nc.tensor.matmul(ps_ap, lh, rh, start=(i == 0),
                                     stop=(i == n - 1))

            # ---------- attention-score phase for step t (emitted in the
            # tail of step t-1; depends only on hTs) ----------
            def attn_phase(dec):
                """Returns dict with qw diag tiles + iwT for the next step."""
                r = {}
                # a = ak(h) + akb (batch-major), aT
                a_bf = st.tile([128, 2, K], BF16, tag="a_bf", name="a_bf")
                aT = st.tile([128, BS], BF16, tag="aT", name="aT")
                for bt in range(2):
                    pb, off = PBS[bt], BOFF[bt]
                    sl = slice(off, off + pb)
                    psa = psp.tile([128, K], F32, tag="gate", name="psa")
                    pairs = [(hTs[kt][:, sl], s_ak[:, kt, :])
                             for kt in range(4)]
                    pairs.append((ones_bf[0:1, :pb], s_akb[:]))
                    emit_group(psa[:pb, :], pairs)
                    nc.scalar.copy(a_bf[:pb, bt, :], psa[:pb, :])
                    tr(aT[:K, off:off + pb], a_bf[:pb, bt, :], pb, K)
                if not dec:
                    return r
                # question attention softmax -> normalized diag tiles.
                # dg2[b', b, l] = ew[b', l] * rs[b'] * I[b', b] in ONE
                # scalar_tensor_tensor (2x mode: all innermost stride-1).
                dg2 = wk.tile([128, 128, L], BF16, tag="dg2", bufs=2,
                              name="dg2")
                dg1b = wk.tile([128, 32, 5], BF16, tag="dg1b", bufs=2,
                              name="dg1b")
                ews = []
                rss = []
                for bt in range(2):
                    pb = PBS[bt]
                    prod = wk.tile([128, L, K], BF16, tag="prod", bufs=2,
                                   name="prod")
                    nc.vector.tensor_mul(prod[:pb], qkbs[bt][:pb],
                                         bcast_mid(a_bf[:pb, bt, :], L))
                    qe = st.tile([128, L], BF16, tag="qe", name="qe")
                    with nc.allow_low_precision(reason="attn scores bf16"):
                        nc.vector.tensor_reduce(qe[:pb], prod[:pb],
                                                axis=AX.X, op=ALU.add)
                    nc.vector.tensor_add(qe[:pb], qe[:pb], s_qem[:pb, bt, :])
                    nm = st.tile([128, 1], F32, tag="nm", name="nm")
                    nc.vector.tensor_reduce(nm[:pb], qe[:pb], axis=AX.X,
                                            op=ALU.max, negate=True)
                    ew = st.tile([128, L], BF16, tag="ew", name="ew")
                    ssum = st.tile([128, 1], F32, tag="ssum", name="ssum")
                    nc.scalar.activation(ew[:pb], qe[:pb], AF.Exp,
                                         bias=nm[:pb], scale=1.0,
                                         accum_out=ssum[:pb])
                    rs = st.tile([128, 1], F32, tag="rs", name="rs")
                    nc.vector.reciprocal(rs[:pb], ssum[:pb])
                    ews.append(ew)
                    rss.append(rs)
                qwn = st.tile([128, L], BF16, tag="qwn", name="qwn")
                nc.vector.tensor_scalar_mul(qwn[:128, :], ews[0][:128, :],
                                            rss[0][:128, :])
                nc.vector.tensor_mul(dg2[:, :, :],
                                     bcast_mid(qwn[:128, :], 128),
                                     i_rep[:, :, :])
                ew_pk = st.tile([128, 5], BF16, tag="ew_pk", name="ew_pk")
                for g4 in range(4):
                    nc.vector.tensor_scalar_mul(
                        ew_pk[32 * g4:32 * (g4 + 1), :],
                        ews[1][0:32, g4:L:4], rss[1][0:32, :])
                nc.vector.tensor_mul(dg1b[:, :, :],
                                     bcast_mid(ew_pk[:, :], 32),
                                     sid_rep[:, :, :])
                r["dg"] = (dg2, dg1b)
                # image attention softmax -> iwT
                iwT = st.tile([128, 2, BS], BF16, tag="iwT", name="iwT")
                for bt in range(2):
                    pb, off = PBS[bt], BOFF[bt]
                    psi = psp.tile([128, 256], F32, tag="gate", name="psi")
                    nc.tensor.matmul(psi[:pb, :], aT[:K, off:off + pb],
                                     ikt2[:K, :, :], start=True, stop=True)
                    iem = st.tile([128, 256], BF16, tag="iem", name="iem")
                    with nc.allow_low_precision(reason="attn scores bf16"):
                        nc.vector.tensor_add(iem[:pb], psi[:pb],
                                             s_iem[:pb, bt, :])
                    nmi = st.tile([128, 1], F32, tag="nmi", name="nmi")
                    nc.vector.tensor_reduce(nmi[:pb], iem[:pb], axis=AX.X,
                                            op=ALU.max, negate=True)
                    ewi = st.tile([128, 256], BF16, tag="ewi", name="ewi")
                    ssi = st.tile([128, 1], F32, tag="ssi", name="ssi")
                    nc.scalar.activation(ewi[:pb], iem[:pb], AF.Exp,
                                         bias=nmi[:pb], scale=1.0,
                                         accum_out=ssi[:pb])
                    rsi = st.tile([128, 1], F32, tag="rsi", name="rsi")
                    nc.vector.reciprocal(rsi[:pb], ssi[:pb])
                    drs = st.tile([128, 128], BF16, tag="drs", name="drs")
                    nc.vector.tensor_scalar_mul(drs[:pb, :pb],
                                                ident_bf[:pb, :pb],
                                                rsi[:pb])
                    for c in range(2):
                        p = psp.tile([128, 128], F32, tag="trp", bufs=2,
                                     name="ptw")
                        nc.tensor.matmul(p[:128, :pb],
                                         ewi[:pb, c * 128:(c + 1) * 128],
                                         drs[:pb, :pb],
                                         start=True, stop=True)
                        nc.vector.tensor_copy(iwT[:, c, off:off + pb],
                                              p[:128, :pb])
                r["iwT"] = iwT
                return r

            # ---------- feature-major GRU core ----------
            def gru_bn(w_gh_s, bhh_s, act_copies=False):
                """BN wave: gh_n x h + bhh_n. Depends only on hTs — emit as
                early as possible in the step."""
                bn_ps = []
                for ht in range(4):
                    sl = slice(2 * H + 128 * ht, 2 * H + 128 * (ht + 1))
                    ps = psp.tile([128, BS], F32, tag="gate", name="bn")
                    pairs = [(w_gh_s[:, kt, sl], hTs[kt][:, :])
                             for kt in range(4)]
                    pairs.append((bhh_s[0:1, 128 * ht:128 * (ht + 1)],
                                  ones_bf[0:1, :]))
                    emit_group(ps[:, :], pairs)
                    bn_ps.append(ps)
                bnF = [st.tile([128, BS], F32, tag=f"bnF{ht}", bufs=1,
                               name="bnF") for ht in range(4)]
                for ht in range(4):
                    (nc.scalar.copy if (act_copies or ht % 2) else
                     nc.vector.tensor_copy)(bnF[ht][:], bn_ps[ht][:, :])
                return bnF

            def gru_fm(w_gi_s, w_gh_s, bhh_s, xt, xrows, extra, bnF,
                       x_late=False):
                """extra: list of (sbuf_tile_or_list, kt_base). Updates
                hTs/hFs in place. Gate math pipelined per h-tile."""
                def xa(tile_sb, k):
                    if isinstance(tile_sb, list):
                        return tile_sb[k][:, 0:BS]
                    return tile_sb[:, k, 0:BS]

                def gate_wave(ci):
                    tiles = []
                    for ht in range(4):
                        sl = slice(ci * H + 128 * ht, ci * H + 128 * (ht + 1))
                        ps = psp.tile([128, BS], F32, tag="gate",
                                      name=f"g{ci}")
                        pairs = []
                        if ci < 2:
                            pairs += [(w_gh_s[:, kt, sl], hTs[kt][:, :])
                                      for kt in range(4)]
                        xpairs = [(w_gi_s[:nr, kt, sl], xt[0:nr, kt, 0:BS])
                                  for kt, nr in enumerate(xrows)]
                        if not x_late:
                            pairs += xpairs
                        for (tile_sb, ktb) in extra:
                            for k in range(4):
                                pairs.append((w_gi_s[:, ktb + k, sl],
                                              xa(tile_sb, k)))
                        if x_late:
                            pairs += xpairs
                        emit_group(ps[:, :], pairs)
                        tiles.append(ps)
                    return tiles

                r_ps = gate_wave(0)
                rF = [st.tile([128, BS], F32, tag=f"rF{ht}", bufs=1,
                              name="rF") for ht in range(4)]
                for ht in range(4):
                    nc.scalar.activation(rF[ht][:], r_ps[ht][:, :],
                                         AF.Sigmoid)
                z_ps = gate_wave(1)
                zF = [st.tile([128, BS], F32, tag=f"zF{ht}", bufs=1,
                              name="zF") for ht in range(4)]
                for ht in range(4):
                    nc.scalar.activation(zF[ht][:], z_ps[ht][:, :],
                                         AF.Sigmoid)
                n_ps = gate_wave(2)
                # per-tile chains: t1 = r*bn + n_ps; n = tanh(t1);
                # h' = n + z*(h-n); hT = bf16(h')
                for ht in range(4):
                    t1 = st.tile([128, BS], F32, tag=f"t1{ht}", bufs=1,
                                 name="t1")
                    nc.vector.tensor_mul(t1[:], rF[ht][:], bnF[ht][:])
                    nc.vector.tensor_add(t1[:], t1[:], n_ps[ht][:, :])
                    nF = st.tile([128, BS], F32, tag=f"nF{ht}", bufs=1,
                                 name="nF")
                    nc.scalar.activation(nF[:], t1[:], AF.Tanh)
                    dd = st.tile([128, BS], F32, tag=f"dd{ht}", bufs=1,
                                 name="dd")
                    eng = nc.gpsimd if ht % 2 else nc.vector
                    eng.tensor_sub(dd[:], hFs[ht][:], nF[:])
                    eng.tensor_mul(dd[:], dd[:], zF[ht][:])
                    eng.tensor_add(hFs[ht][:], dd[:], nF[:])
                    nc.vector.tensor_copy(hTs[ht][:], hFs[ht][:])

            # ---------- image projections ----------
            for mt in range(2):
                psv = psp.tile([128, H], F32, tag="gate", name="psv")
                emit_group(psv[:], [(s_imgt[:, kt, mt * 128:(mt + 1) * 128],
                                     s_iv[:, kt, :]) for kt in range(2)])
                nc.scalar.copy(ivv[:, mt, :], psv[:])
            for mt in range(2):
                psik = psp.tile([128, 128], F32, tag="gate", name="psik")
                emit_group(psik[:K, :],
                           [(s_ik[:, kt, :],
                             s_imgt[:, kt, mt * 128:(mt + 1) * 128])
                            for kt in range(2)])
                nc.vector.tensor_copy(ikt2[:K, mt, :], psik[:K, :])

            # ---------- encoder ----------
            with tc.tile_pool(name="qp", bufs=1) as qp:
                s_egi = load(qp, w_egi, BF16)
                s_egh = load(qp, w_egh, BF16)
                xt_q = fetch_x(s_qidx, 0)
                # decoder weights: prefetch now, overlapping encoder compute
                s_gi = load(pers, w_gi, BF16)
                s_gh = load(pers, w_gh, BF16)
                s_out = load(pers, w_out, BF16)
                for t in range(L):
                    bnF = gru_bn(s_egh, s_ebhh)
                    gru_fm(s_egi, s_egh, s_ebhh, xt_q, [128, 128, 65], [],
                           bnF)
                    if t + 1 < L:
                        xt_q = fetch_x(s_qidx, t + 1)
                    for bt in range(2):
                        pb, off = PBS[bt], BOFF[bt]
                        sl = slice(off, off + pb)
                        psk = psp.tile([128, K], F32, tag="gate", name="psk")
                        emit_group(psk[:pb, :],
                                   [(hTs[kt][:, sl], s_qk[:, kt, :])
                                    for kt in range(4)])
                        nc.scalar.copy(qkbs[bt][:pb, t, :], psk[:pb, :])
                        psv = psp.tile([128, H], F32, tag="gate", name="psv")
                        emit_group(psv[:pb, :],
                                   [(hTs[kt][:, sl], s_qv[:, kt, :])
                                    for kt in range(4)])
                        if bt == 0:
                            nc.scalar.copy(qv_b0[:pb, t, :], psv[:pb, :])
                        else:
                            g4 = t % 4
                            nc.scalar.copy(
                                qv_p1[32 * g4:32 * (g4 + 1), t // 4, :],
                                psv[:pb, :])

            for i in range(4):
                nc.vector.memset(hTs[i][:], 0.0)
                nc.vector.memset(hFs[i][:], 0.0)

            # ---------- decoder ----------
            with tc.tile_pool(name="lg", bufs=1) as lg:
                o19T = lg.tile([128, 3, BS], FP8)
                nc.vector.memset(o19T[32:64, 2, :], 0.0)
                nc.vector.memset(o19T[64:65, 2, :], 1.0)
                o19_0 = lg.tile([128, D], F32)
                o19_1 = lg.tile([128, D], F32)
                o19_sb = [o19_0, o19_1]

                xt_a = fetch_x(s_aidx, 0)
                at = attn_phase(True)   # for step 0
                for t in range(MAX_LEN):
                    dg2, dg1b = at["dg"]
                    iwT = at["iwT"]

                    bnF = gru_bn(s_gh, s_bhh, act_copies=(t == L))

                    # --- icT (feature-major) ---
                    icT = [wk.tile([128, BS], BF16, tag=f"icT{k}", bufs=2,
                                   name="icT") for k in range(4)]
                    for ht in range(4):
                        ps = psp.tile([128, BS], F32, tag="gate", name="psic")
                        emit_group(ps[:, :],
                                   [(ivv[:, mt, 128 * ht:128 * (ht + 1)],
                                     iwT[:, mt, :]) for mt in range(2)])
                        nc.scalar.activation(icT[ht][:], ps[:, :],
                                             AF.Identity,
                                             bias=s_ivb[:, ht:ht + 1],
                                             scale=1.0)

                    # --- qcT (feature-major, symmetric diag trick) ---
                    qcT = [wk.tile([128, BS], BF16, tag=f"qcT{k}", bufs=2,
                                   name="qcT") for k in range(4)]
                    for ht in range(4):
                        hsl = slice(128 * ht, 128 * (ht + 1))
                        ps = psp.tile([128, BS], F32, tag="gate", name="psqc")
                        first = True
                        for l in range(L):
                            nc.tensor.matmul(ps[:, 0:128],
                                             qv_b0[:, l, hsl],
                                             dg2[:, :, l:l + 1],
                                             start=first, stop=False)
                            first = False
                        for c in range(5):
                            nc.tensor.matmul(ps[:, 128:BS],
                                             qv_p1[:, c, hsl],
                                             dg1b[:, :, c:c + 1],
                                             start=False, stop=(c == 4))
                        if ht % 2 or t == L:
                            nc.scalar.activation(qcT[ht][:], ps[:, :],
                                                 AF.Identity,
                                                 bias=s_qvb[:, ht:ht + 1],
                                                 scale=1.0)
                        else:
                            nc.vector.tensor_scalar_add(qcT[ht][:], ps[:, :],
                                                        s_qvb[:, ht:ht + 1])

                    # --- GRU ---
                    xsrc = xt_a if t < L else dec20
                    gru_fm(s_gi, s_gh, s_bhh, xsrc, [128, 128, 65],
                           [(icT, 7), (qcT, 3)], bnF, x_late=(t == L))
                    if t < L - 1:
                        xt_a = fetch_x(s_aidx, t + 1)

                    # --- attention phase for next step (overlaps out-proj) ---
                    if t + 1 < MAX_LEN:
                        at = attn_phase(True)

                    # --- output projection ---
                    if t != MAX_LEN - 2:
                        osbT = st.tile([128, 3, BS], F32, tag="osbT",
                                       name="osbT")
                        nc.vector.memset(osbT[32:64, 2, :], 0.0)
                        nc.vector.memset(osbT[64:, 2, :], 0.0)
                        for dt_ in range(3):
                            nd = 128 if dt_ < 2 else D - 256
                            sl = slice(128 * dt_, 128 * dt_ + nd)
                            ps = psp.tile([128, BS], F32, tag="gate",
                                          name="pso")
                            pairs = [(s_out[:, 4 + k, sl], qcT[k][:, 0:BS])
                                     for k in range(4)]
                            pairs += [(s_out[:, 8 + k, sl], icT[k][:, 0:BS])
                                      for k in range(4)]
                            pairs.append((s_outb[0:1, sl], ones_bf[0:1, :]))
                            pairs += [(s_out[:, k, sl], hTs[k][:, :])
                                      for k in range(4)]
                            n = len(pairs)
                            for i, (lh, rh) in enumerate(pairs):
                                nc.tensor.matmul(ps[:nd, :], lh, rh,
                                                 start=(i == 0),
                                                 stop=(i == n - 1))
                            nc.vector.tensor_copy(osbT[:nd, dt_, :],
                                                  ps[:nd, :])
                        nc.sync.dma_start(out_o[t], osbT[:])
                    else:
                        # t == 19: batch-major out for argmax rescoring
                        for bt in range(2):
                            pb, off = PBS[bt], BOFF[bt]
                            sl = slice(off, off + pb)
                            pso = psp.tile([128, D], F32, tag="gate",
                                           name="pso19")
                            pairs = [(hTs[k][:, sl], s_out[:, k, :])
                                     for k in range(4)]
                            pairs += [(qcT[k][:, sl], s_out[:, 4 + k, :])
                                      for k in range(4)]
                            pairs += [(icT[k][:, sl], s_out[:, 8 + k, :])
                                      for k in range(4)]
                            pairs.append((ones_bf[0:1, :pb], s_outb[:]))
                            emit_group(pso[:pb, :], pairs)
                            osb = o19_sb[bt]
                            nc.scalar.copy(osb[:pb], pso[:pb])
                            nc.sync.dma_start(out_o19[:pb, bt, :], osb[:pb])
                            ob = st.tile([128, D], FP8, tag="ob", name="ob")
                            nc.scalar.copy(ob[:pb], osb[:pb])
                            for kt in range(3):
                                w = 128 if kt < 2 else D - 256
                                p8t = psp.tile([128, 128], FP8, tag="trp",
                                               bufs=2, name="p8t")
                                nc.tensor.transpose(p8t[:w, :pb],
                                                    ob[:pb,
                                                       kt * 128:kt * 128 + w],
                                                    ident_f8[:pb, :pb])
                                nc.vector.tensor_copy(
                                    o19T[:w, kt, off:off + pb], p8t[:w, :pb])

                    # --- argmax + re-embed for step 20 ---
                    if t == MAX_LEN - 2:
                        mx18 = [lg.tile([128, 144], F32, name="mx18a"),
                                lg.tile([128, 144], F32, name="mx18b")]
                        ix18 = [lg.tile([128, 144], F32, name="ix18a"),
                                lg.tile([128, 144], F32, name="ix18b")]
                        for nci in range(18):
                            ncw = 512 if nci < 17 else V - 17 * 512
                            rhs = wk.tile([128, 3, 512], FP8, tag="lrhs",
                                          bufs=4, name="rhs")
                            for kt in range(3):
                                nr = 128 if kt < 2 else 65
                                nc.sync.dma_start(
                                    rhs[:nr, kt, :ncw],
                                    embt_bf[:nr, kt,
                                            nci * 512:nci * 512 + ncw])
                            for bt in range(2):
                                pb, off = PBS[bt], BOFF[bt]
                                psl = psp.tile([128, H], F32, tag="gate",
                                               name="psl")
                                if ncw < 512:
                                    nc.vector.memset(psl[:pb, ncw:], NEG)
                                nc.tensor.matmul(
                                    psl[:pb, :ncw],
                                    o19T[:, 0:2, off:off + pb],
                                    rhs[:, 0:2, :ncw],
                                    start=True, stop=False,
                                    perf_mode=PM.DoubleRow)
                                nc.tensor.matmul(
                                    psl[:pb, :ncw],
                                    o19T[:65, 2, off:off + pb],
                                    rhs[:65, 2, :ncw],
                                    start=False, stop=True)
                                nc.vector.max(
                                    mx18[bt][:pb, 8 * nci:8 * nci + 8],
                                    psl[:pb, :])
                                ixc = st.tile([128, 8], U32, tag="ixc",
                                              bufs=3, name="ixc")
                                nc.vector.max_index(
                                    ixc[:pb],
                                    mx18[bt][:pb, 8 * nci:8 * nci + 8],
                                    psl[:pb, :])
                                nc.vector.tensor_copy(
                                    ix18[bt][:pb, 8 * nci:8 * nci + 8],
                                    ixc[:pb])
                        g8s = {}
                        for bt in range(2):
                            pb, off = PBS[bt], BOFF[bt]
                            ixg = st.tile([128, 144], F32, tag=f"ixg{bt}",
                                          name="ixg")
                            nc.vector.tensor_add(ixg[:pb], ix18[bt][:pb],
                                                 off18[:pb])
                            c8 = st.tile([128, 8], F32, tag=f"c8{bt}",
                                         name="c8")
                            nc.vector.max(c8[:pb], mx18[bt][:pb])
                            p8 = st.tile([128, 8], U32, tag=f"p8{bt}",
                                         name="p8")
                            nc.vector.max_index(p8[:pb], c8[:pb],
                                                mx18[bt][:pb])
                            p8f = st.tile([128, 8], F32, tag=f"p8f{bt}",
                                          name="p8f")
                            nc.vector.tensor_copy(p8f[:pb], p8[:pb])
                            ix8f = st.tile([128, 8], F32, tag=f"ix8f{bt}",
                                           name="ix8f")
                            g8s[bt] = []
                            for j in range(8):
                                oh = wk.tile([128, 144], F32, tag="oh144",
                                             bufs=2, name="oh")
                                nc.vector.tensor_scalar(
                                    out=oh[:pb], in0=iota144[:pb],
                                    scalar1=p8f[:pb, j:j + 1], scalar2=None,
                                    op0=ALU.is_equal)
                                nc.vector.tensor_mul(oh[:pb], oh[:pb],
                                                     ixg[:pb])
                                vj = st.tile([128, 1], F32, tag="vj",
                                             name="vj")
                                nc.vector.tensor_reduce(vj[:pb], oh[:pb],
                                                        axis=AX.X, op=ALU.add)
                                nc.vector.tensor_copy(ix8f[:pb, j:j + 1],
                                                      vj[:pb])
                                vju = st.tile([128, 1], U32, tag="vju",
                                              name="vju")
                                nc.vector.tensor_copy(vju[:pb], vj[:pb])
                                g8 = wk.tile([128, D + 1], F32, tag="gath8",
                                             bufs=10, name="g8")
                                nc.gpsimd.indirect_dma_start(
                                    out=g8[:pb], out_offset=None,
                                    in_=emb_aug[:],
                                    in_offset=bass.IndirectOffsetOnAxis(
                                        ap=vju[:pb, 0:1], axis=0))
                                g8s[bt].append(g8)
                            g8s[bt + 2] = ix8f
                        for bt in range(2):
                            pb, off = PBS[bt], BOFF[bt]
                            ix8f = g8s[bt + 2]
                            scores = st.tile([128, 8], F32, tag=f"sc8{bt}",
                                             name="scores")
                            for j in range(8):
                                g8 = g8s[bt][j]
                                pr = wk.tile([128, D], F32, tag="pr8",
                                             name="pr")
                                nc.vector.tensor_mul(pr[:pb], o19_sb[bt][:pb],
                                                     g8[:pb, :D])
                                sj = st.tile([128, 1], F32, tag="sj",
                                             name="sj")
                                nc.vector.tensor_reduce(sj[:pb], pr[:pb],
                                                        axis=AX.X, op=ALU.add)
                                nc.vector.tensor_add(scores[:pb, j:j + 1],
                                                     sj[:pb],
                                                     g8[:pb, D:D + 1])
                            m1 = st.tile([128, 8], F32, tag="m1", name="m1")
                            nc.vector.max(m1[:pb], scores[:pb])
                            j1 = st.tile([128, 8], U32, tag="j1", name="j1")
                            nc.vector.max_index(j1[:pb], m1[:pb],
                                                scores[:pb])
                            j1f = st.tile([128, 1], F32, tag="j1f",
                                          name="j1f")
                            nc.vector.tensor_copy(j1f[:pb], j1[:pb, 0:1])
                            oh8 = st.tile([128, 8], F32, tag="oh8",
                                          name="oh8")
                            nc.vector.tensor_scalar(out=oh8[:pb],
                                                    in0=iota8[:pb],
                                                    scalar1=j1f[:pb],
                                                    scalar2=None,
                                                    op0=ALU.is_equal)
                            nc.vector.tensor_mul(ix8f[:pb], oh8[:pb],
                                                 ix8f[:pb])
                            vsum = st.tile([128, 1], F32, tag="vsum",
                                           name="vsum")
                            nc.vector.tensor_reduce(vsum[:pb], ix8f[:pb],
                                                    axis=AX.X, op=ALU.add)
                            vidx = st.tile([128, 1], U32, tag="vidx",
                                           name="vidx")
                            nc.vector.tensor_copy(vidx[:pb], vsum[:pb])
                            gm = wk.tile([128, D], BF16, tag="gath", bufs=6,
                                         name="gm")
                            nc.gpsimd.indirect_dma_start(
                                out=gm[:pb], out_offset=None, in_=emb_bf[:],
                                in_offset=bass.IndirectOffsetOnAxis(
                                    ap=vidx[:pb, 0:1], axis=0))
                            for kt in range(3):
                                w = 128 if kt < 2 else D - 256
                                tr(dec20[:w, kt, off:off + pb],
                                   gm[:pb, kt * 128:kt * 128 + w], pb, w)

    nc.compile()
    return nc


_NC_CACHE = None


def _get_nc():
    global _NC_CACHE
    if _NC_CACHE is None:
        _NC_CACHE = build_nc()
    return _NC_CACHE


def _pad_tiles(a, ntiles):
    rows, cols = a.shape
    out = np.zeros((128 * ntiles, cols), a.dtype)
    out[:rows] = a
    return np.ascontiguousarray(
        out.reshape(ntiles, 128, cols).transpose(1, 0, 2))


def _prep_shared(inputs):
    bf = np.float16
    f32 = np.float32
    eW = np.asarray(inputs["embed_W"], f32)
    d = {}
    wih = np.asarray(inputs["dec_W_ih"], f32)
    bih = np.asarray(inputs["dec_b_ih"], f32)
    bhh = np.asarray(inputs["dec_b_hh"], f32)
    gi = np.zeros((128 * 11, 3 * H), f32)
    gi[0:D] = wih[:, 0:D].T
    gi[320] = bih + np.concatenate([bhh[:2 * H], np.zeros(H, f32)])
    gi[384:384 + H] = wih[:, D:D + H].T
    gi[896:896 + H] = wih[:, D + H:].T
    d["w_gi"] = _pad_tiles(gi.astype(bf), 11)
    d["w_gh"] = _pad_tiles(np.asarray(inputs["dec_W_hh"], f32).T.astype(bf), 4)
    d["bhh_n"] = np.ascontiguousarray(bhh[2 * H:].astype(bf)[None, :])
    ewih = np.asarray(inputs["enc_W_ih"], f32)
    ebih = np.asarray(inputs["enc_b_ih"], f32)
    ebhh = np.asarray(inputs["enc_b_hh"], f32)
    egi = np.zeros((128 * 3, 3 * H), f32)
    egi[0:D] = ewih[:, :D].T
    egi[320] = ebih + np.concatenate([ebhh[:2 * H], np.zeros(H, f32)])
    d["w_egi"] = _pad_tiles(egi.astype(bf), 3)
    d["w_egh"] = _pad_tiles(np.asarray(inputs["enc_W_hh"], f32).T.astype(bf), 4)
    d["ebhh_n"] = np.ascontiguousarray(ebhh[2 * H:].astype(bf)[None, :])
    d["w_out"] = _pad_tiles(np.asarray(inputs["out_W"], f32).T.astype(bf), 12)
    d["outb"] = np.ascontiguousarray(
        np.asarray(inputs["out_b"], f32).astype(bf)[None, :])
    d["w_qk"] = _pad_tiles(np.asarray(inputs["qk_W"], f32).T.astype(bf), 4)
    d["w_qv"] = _pad_tiles(np.asarray(inputs["qv_W"], f32).T.astype(bf), 4)
    d["qvb_c"] = np.ascontiguousarray(
        np.asarray(inputs["qv_b"], f32).reshape(4, 128).T)
    d["w_ak"] = _pad_tiles(np.asarray(inputs["ak_W"], f32).T.astype(bf), 4)
    d["akb"] = np.ascontiguousarray(
        np.asarray(inputs["ak_b"], f32).astype(bf)[None, :])
    d["w_ik"] = _pad_tiles(np.asarray(inputs["ik_W"], f32).T.astype(bf), 2)
    d["w_iv"] = _pad_tiles(np.asarray(inputs["iv_W"], f32).T.astype(bf), 2)
    d["ivb_c"] = np.ascontiguousarray(
        np.asarray(inputs["iv_b"], f32).reshape(4, 128).T)
    d["emb_bf"] = eW.astype(bf)
    wd_b = np.asarray(inputs["wd_b"], f32)
    d["emb_aug"] = np.ascontiguousarray(
        np.concatenate([eW, wd_b[:, None]], 1))
    aug = np.zeros((128 * 3, VP), f32)
    aug[:D, :V] = eW.T
    aug[320, :V] = wd_b
    import ml_dtypes
    d["embt_bf"] = _pad_tiles(aug.astype(ml_dtypes.float8_e4m3fn), 3)
    return d


def _idx_cols(seq_rows):
    out = np.zeros((128, 2 * L), np.uint32)
    for t in range(L):
        out[:, 2 * t] = seq_rows[0:128, t]
        out[:32, 2 * t + 1] = seq_rows[128:160, t]
    return out


def _build_maps(inputs, shared):
    f32 = np.float32
    bf = np.float16
    ques = np.asarray(inputs["ques_seqs"]).astype(np.uint32)
    ans = np.asarray(inputs["ans_seqs"]).astype(np.uint32)
    qlens = np.asarray(inputs["ques_lens"]).astype(np.int64)
    img = np.asarray(inputs["img_seqs"], f32)
    maps = []
    for s in range(NCORES):
        m = dict(shared)
        r0 = s * BS
        m["q_idx"] = _idx_cols(ques[r0:r0 + BS, :L])
        m["a_idx"] = _idx_cols(ans[r0:r0 + BS, :L])
        qm = np.full((128, 2, L), NEG, f32)
        lens = qlens[r0:r0 + BS]
        for bt, (pb, off) in enumerate(zip(PBS, BOFF)):
            for b in range(pb):
                qm[b, bt, :lens[off + b]] = 0.0
        m["qe_mask"] = qm.astype(bf)
        im = np.full((128, 2, 256), NEG, f32)
        for bt, (pb, off) in enumerate(zip(PBS, BOFF)):
            for b in range(pb):
                gimg = (off + b) // ROUNDS
                im[b, bt, gimg * 16:(gimg + 1) * 16] = 0.0
        m["ie_mask"] = im.astype(bf)
        imgs = img[s * 16:(s + 1) * 16].reshape(256, 256)
        it = np.zeros((128 * 2, 256), f32)
        it[:256] = imgs.T
        m["img_t"] = np.ascontiguousarray(
            it.reshape(2, 128, 256).transpose(1, 0, 2)).astype(bf)
        maps.append(m)
    return maps


def kernel(**inputs):
    nc = _get_nc()
    shared = _prep_shared(inputs)
    in_maps = _build_maps(inputs, shared)
    from concourse.bass_utils import run_bass_kernel_spmd
    res = run_bass_kernel_spmd(nc, in_maps, core_ids=list(range(NCORES)))
    outs = []
    for s in range(NCORES):
        o = np.asarray(res.results[s]["out_o"])      # [21, 128, 3, 160]
        o19 = np.asarray(res.results[s]["out_o19"])  # [128, 2, 300]
        full = o.transpose(3, 0, 2, 1).reshape(BS, MAX_LEN, 384)[:, :, :D]
        full = np.ascontiguousarray(full)
        full[:128, MAX_LEN - 2] = o19[:, 0, :]
        full[128:, MAX_LEN - 2] = o19[:32, 1, :]
        outs.append(full)
    return np.concatenate(outs, 0).astype(np.float32)


# revision 4
# speedup vs baseline: 1.0368x; 1.0368x over previous
"""Trainium2 Bass kernel for nn_BaselineAttnDecoder — feature-major,
software-pipelined.

Per core: 160 decode rows (16 images x 10 rounds), weights replicated.

- GRU gates / out-proj / icT / qcT all FEATURE-major: out [feat<=128, 160]
  PSUM groups, moving dim = true batch 160 (no 128+32 padding waste).
- h lives only as per-tile hT (bf16) + hF (f32): no h transposes.
- qcT via symmetric diag-trick (lhsT = batch-major q_value, rhs = the
  qw diagonal) — attention-weighted sum lands directly feature-major.
- Attention score chain (a, qe, iw softmax, diag build) for step t+1 is
  emitted in step t's tail so the DVE chain overlaps PE matmul work.
- Gate elementwise math runs per h-tile, pipelined across Act/DVE/Pool.
- Step-19 vocab argmax: blockwise top-8 from PSUM chunks (bf16 copies on
  Act, Max/MaxIndex on DVE), exact-f32 rescore of 8 candidates.
"""
import numpy as np

import concourse.bass as bass
import concourse.bacc as bacc
import concourse.mybir as mybir
import concourse.tile as tile
from concourse.masks import make_identity

F32 = mybir.dt.float32
BF16 = mybir.dt.float16  # 16-bit compute dtype (f16: 10-bit mantissa)
U32 = mybir.dt.uint32
FP8 = mybir.dt.float8e4
PM = mybir.MatmulPerfMode
AF = mybir.ActivationFunctionType
ALU = mybir.AluOpType
AX = mybir.AxisListType

D, H, V, K = 300, 512, 8835, 50
L, MAX_LEN, ROUNDS = 20, 21, 10
BS = 160
NCORES = 8
PBS = [128, 32]
BOFF = [0, 128]
VP = 18 * 512
NEG = -60000.0


def bcast_mid(ap, reps):
    return bass.AP(tensor=ap.tensor, offset=ap.offset,
                   ap=[ap.ap[0], [0, reps], ap.ap[1]])


def bcast_in(ap, reps):
    return bass.AP(tensor=ap.tensor, offset=ap.offset,
                   ap=[ap.ap[0], ap.ap[1], [0, reps]])


def build_nc():
    nc = bacc.Bacc()

    def din(name, shape, dt):
        return nc.dram_tensor(name, shape, dt, kind="ExternalInput")

    w_gi = din("w_gi", [128, 11, 3 * H], BF16)
    w_gh = din("w_gh", [128, 4, 3 * H], BF16)
    bhh_n = din("bhh_n", [1, H], BF16)
    w_egi = din("w_egi", [128, 3, 3 * H], BF16)
    w_egh = din("w_egh", [128, 4, 3 * H], BF16)
    ebhh_n = din("ebhh_n", [1, H], BF16)
    w_out = din("w_out", [128, 12, D], BF16)
    outb = din("outb", [1, D], BF16)
    w_qk = din("w_qk", [128, 4, K], BF16)
    w_qv = din("w_qv", [128, 4, H], BF16)
    w_ak = din("w_ak", [128, 4, K], BF16)
    akb = din("akb", [1, K], BF16)
    w_ik = din("w_ik", [128, 2, K], BF16)
    w_iv = din("w_iv", [128, 2, H], BF16)
    ivb_c = din("ivb_c", [128, 4], F32)
    qvb_c = din("qvb_c", [128, 4], F32)
    img_t = din("img_t", [128, 2, 2 * 128], BF16)
    emb_bf = din("emb_bf", [V, D], BF16)
    emb_aug = din("emb_aug", [V, D + 1], F32)
    embt_bf = din("embt_bf", [128, 3, VP], FP8)
    q_idx = din("q_idx", [128, 2 * L], U32)
    a_idx = din("a_idx", [128, 2 * L], U32)
    qe_mask = din("qe_mask", [128, 2, L], BF16)
    ie_mask = din("ie_mask", [128, 2, 2 * 128], BF16)

    out_o = nc.dram_tensor("out_o", [MAX_LEN, 128, 3, BS], F32,
                           kind="ExternalOutput")
    out_o19 = nc.dram_tensor("out_o19", [128, 2, D], F32,
                             kind="ExternalOutput")

    with tile.TileContext(nc) as tc:
        with (
            tc.tile_pool(name="cw", bufs=1) as cw,
            tc.tile_pool(name="pers", bufs=1) as pers,
            tc.tile_pool(name="wk", bufs=2) as wk,
            tc.tile_pool(name="st", bufs=2) as st,
            tc.tile_pool(name="ps", bufs=5, space="PSUM") as psp,
        ):
            def load(pool, t, dt):
                s = pool.tile(list(t.shape), dt, name=t.name + "_sb")
                nc.sync.dma_start(s[:], t[:])
                return s

            s_qk = load(cw, w_qk, BF16)
            s_qv = load(cw, w_qv, BF16)
            s_ak = load(cw, w_ak, BF16)
            s_ik = load(cw, w_ik, BF16)
            s_iv = load(cw, w_iv, BF16)
            s_imgt = load(cw, img_t, BF16)
            s_bhh = load(cw, bhh_n, BF16)
            s_ebhh = load(cw, ebhh_n, BF16)
            s_outb = load(cw, outb, BF16)
            s_akb = load(cw, akb, BF16)
            s_ivb = load(cw, ivb_c, F32)
            s_qvb = load(cw, qvb_c, F32)
            s_qidx = load(cw, q_idx, U32)
            s_aidx = load(cw, a_idx, U32)
            s_qem = load(cw, qe_mask, BF16)
            s_iem = load(cw, ie_mask, BF16)

            ident_bf = cw.tile([128, 128], BF16)
            make_identity(nc, ident_bf[:])
            ones_bf = cw.tile([1, BS], BF16)
            nc.vector.memset(ones_bf[:], 1.0)
            sid4 = cw.tile([128, 32], BF16)
            for g4 in range(4):
                nc.vector.tensor_copy(sid4[32 * g4:32 * (g4 + 1), :],
                                      ident_bf[0:32, 0:32])
            iota8 = cw.tile([128, 8], F32)
            nc.gpsimd.iota(iota8[:], pattern=[[1, 8]], base=0,
                           channel_multiplier=0,
                           allow_small_or_imprecise_dtypes=True)
            iota144 = cw.tile([128, 144], F32)
            nc.gpsimd.iota(iota144[:], pattern=[[1, 144]], base=0,
                           channel_multiplier=0,
                           allow_small_or_imprecise_dtypes=True)
            off18 = cw.tile([128, 144], F32)
            nc.gpsimd.iota(off18[:], pattern=[[512, 18], [0, 8]], base=0,
                           channel_multiplier=0,
                           allow_small_or_imprecise_dtypes=True)
            # identity replicated along an inner L/5 axis (for diag builds
            # that keep innermost stride-1 so DVE 2x mode applies)
            i_rep = cw.tile([128, 128, L], BF16)
            for l in range(L):
                nc.vector.tensor_copy(i_rep[:, :, l:l + 1],
                                      bass.AP(tensor=ident_bf.tensor,
                                              offset=ident_bf[:, :].offset,
                                              ap=[ident_bf[:, :].ap[0],
                                                  [1, 128], [0, 1]]))
            sid_rep = cw.tile([128, 32, 5], BF16)
            for c in range(5):
                nc.vector.tensor_copy(sid_rep[:, :, c:c + 1],
                                      bass.AP(tensor=sid4.tensor,
                                              offset=sid4[:, :].offset,
                                              ap=[sid4[:, :].ap[0],
                                                  [1, 32], [0, 1]]))

            # persistent state — h per tile, feature-major
            hTs = [pers.tile([128, BS], BF16, name=f"hT{i}") for i in range(4)]
            hFs = [pers.tile([128, BS], F32, name=f"hF{i}") for i in range(4)]
            qk_b0 = pers.tile([128, L, K], BF16)
            qk_b1 = pers.tile([128, L, K], BF16)
            qkbs = [qk_b0, qk_b1]
            qv_b0 = pers.tile([128, L, H], BF16)
            qv_p1 = pers.tile([128, 5, H], BF16)
            ivv = pers.tile([128, 2, H], BF16)
            ikt2 = pers.tile([128, 2, 128], BF16)
            dec20 = pers.tile([128, 3, BS], BF16)

            for i in range(4):
                nc.vector.memset(hTs[i][:], 0.0)
                nc.vector.memset(hFs[i][:], 0.0)
            nc.vector.memset(dec20[32:64, 2, :], 0.0)
            nc.vector.memset(dec20[64:65, 2, :], 1.0)

            def tr(dst_sb_ap, src_sb_ap, pb, w, eng=None):
                p = psp.tile([128, 128], BF16, tag="trp", bufs=2, name="pt")
                nc.tensor.transpose(p[:w, :pb], src_sb_ap, ident_bf[:pb, :pb])
                (eng or nc.vector).tensor_copy(dst_sb_ap, p[:w, :pb])

            def fetch_x(idx_sb, t):
                xt = wk.tile([128, 3, BS], BF16, tag="xt", bufs=3, name="xt")
                nc.vector.memset(xt[32:64, 2, :], 0.0)
                nc.vector.memset(xt[64:65, 2, :], 1.0)
                for c, (pb, off) in enumerate(zip(PBS, BOFF)):
                    g = wk.tile([128, D], BF16, tag="gath", bufs=6, name="g")
                    nc.gpsimd.indirect_dma_start(
                        out=g[:pb], out_offset=None, in_=emb_bf[:],
                        in_offset=bass.IndirectOffsetOnAxis(
                            ap=idx_sb[:pb, 2 * t + c:2 * t + c + 1], axis=0))
                    for kt in range(3):
                        w = 128 if kt < 2 else D - 256
                        if kt == 1:
                            p = psp.tile([128, 128], BF16, tag="trp", bufs=2,
                                         name="pt")
                            nc.tensor.transpose(p[:w, :pb],
                                                g[:pb, 128:128 + w],
                                                ident_bf[:pb, :pb])
                            nc.scalar.copy(xt[:w, kt, off:off + pb],
                                           p[:w, :pb])
                        else:
                            tr(xt[:w, kt, off:off + pb],
                               g[:pb, kt * 128:kt * 128 + w], pb, w)
                return xt

            def emit_group(ps_ap, pairs):
                n = len(pairs)
                for i, (lh, rh) in enumerate(pairs):
                    nc.tensor.matmul(ps_ap, lh, rh, start=(i == 0),
                                     stop=(i == n - 1))

            # ---------- attention-score phase for step t (emitted in the
            # tail of step t-1; depends only on hTs) ----------
            def attn_phase(dec):
                """Returns dict with qw diag tiles + iwT for the next step."""
                r = {}
                # a = ak(h) + akb (batch-major), aT
                a_bf = st.tile([128, 2, K], BF16, tag="a_bf", name="a_bf")
                aT = st.tile([128, BS], BF16, tag="aT", name="aT")
                for bt in range(2):
                    pb, off = PBS[bt], BOFF[bt]
                    sl = slice(off, off + pb)
                    psa = psp.tile([128, K], F32, tag="gate", name="psa")
                    pairs = [(hTs[kt][:, sl], s_ak[:, kt, :])
                             for kt in range(4)]
                    pairs.append((ones_bf[0:1, :pb], s_akb[:]))
                    emit_group(psa[:pb, :], pairs)
                    nc.scalar.copy(a_bf[:pb, bt, :], psa[:pb, :])
                    tr(aT[:K, off:off + pb], a_bf[:pb, bt, :], pb, K)
                if not dec:
                    return r
                # question attention softmax -> normalized diag tiles.
                # dg2[b', b, l] = ew[b', l] * rs[b'] * I[b', b] in ONE
                # scalar_tensor_tensor (2x mode: all innermost stride-1).
                dg2 = wk.tile([128, 128, L], BF16, tag="dg2", bufs=2,
                              name="dg2")
                dg1b = wk.tile([128, 32, 5], BF16, tag="dg1b", bufs=2,
                              name="dg1b")
                ews = []
                rss = []
                for bt in range(2):
                    pb = PBS[bt]
                    prod = wk.tile([128, L, K], BF16, tag="prod", bufs=2,
                                   name="prod")
                    nc.vector.tensor_mul(prod[:pb], qkbs[bt][:pb],
                                         bcast_mid(a_bf[:pb, bt, :], L))
                    qe = st.tile([128, L], BF16, tag="qe", name="qe")
                    with nc.allow_low_precision(reason="attn scores bf16"):
                        nc.vector.tensor_reduce(qe[:pb], prod[:pb],
                                                axis=AX.X, op=ALU.add)
                    nc.vector.tensor_add(qe[:pb], qe[:pb], s_qem[:pb, bt, :])
                    nm = st.tile([128, 1], F32, tag="nm", name="nm")
                    nc.vector.tensor_reduce(nm[:pb], qe[:pb], axis=AX.X,
                                            op=ALU.max, negate=True)
                    ew = st.tile([128, L], BF16, tag="ew", name="ew")
                    ssum = st.tile([128, 1], F32, tag="ssum", name="ssum")
                    nc.scalar.activation(ew[:pb], qe[:pb], AF.Exp,
                                         bias=nm[:pb], scale=1.0,
                                         accum_out=ssum[:pb])
                    rs = st.tile([128, 1], F32, tag="rs", name="rs")
                    nc.vector.reciprocal(rs[:pb], ssum[:pb])
                    ews.append(ew)
                    rss.append(rs)
                qwn = st.tile([128, L], BF16, tag="qwn", name="qwn")
                nc.vector.tensor_scalar_mul(qwn[:128, :], ews[0][:128, :],
                                            rss[0][:128, :])
                nc.vector.tensor_mul(dg2[:, :, :],
                                     bcast_mid(qwn[:128, :], 128),
                                     i_rep[:, :, :])
                ew_pk = st.tile([128, 5], BF16, tag="ew_pk", name="ew_pk")
                for g4 in range(4):
                    nc.vector.tensor_scalar_mul(
                        ew_pk[32 * g4:32 * (g4 + 1), :],
                        ews[1][0:32, g4:L:4], rss[1][0:32, :])
                nc.vector.tensor_mul(dg1b[:, :, :],
                                     bcast_mid(ew_pk[:, :], 32),
                                     sid_rep[:, :, :])
                r["dg"] = (dg2, dg1b)
                # image attention softmax -> iwT
                iwT = st.tile([128, 2, BS], BF16, tag="iwT", name="iwT")
                for bt in range(2):
                    pb, off = PBS[bt], BOFF[bt]
                    psi = psp.tile([128, 256], F32, tag="gate", name="psi")
                    nc.tensor.matmul(psi[:pb, :], aT[:K, off:off + pb],
                                     ikt2[:K, :, :], start=True, stop=True)
                    iem = st.tile([128, 256], BF16, tag="iem", name="iem")
                    with nc.allow_low_precision(reason="attn scores bf16"):
                        nc.vector.tensor_add(iem[:pb], psi[:pb],
                                             s_iem[:pb, bt, :])
                    nmi = st.tile([128, 1], F32, tag="nmi", name="nmi")
                    nc.vector.tensor_reduce(nmi[:pb], iem[:pb], axis=AX.X,
                                            op=ALU.max, negate=True)
                    ewi = st.tile([128, 256], BF16, tag="ewi", name="ewi")
                    ssi = st.tile([128, 1], F32, tag="ssi", name="ssi")
                    nc.scalar.activation(ewi[:pb], iem[:pb], AF.Exp,
                                         bias=nmi[:pb], scale=1.0,
                                         accum_out=ssi[:pb])
                    rsi = st.tile([128, 1], F32, tag="rsi", name="rsi")
                    nc.vector.reciprocal(rsi[:pb], ssi[:pb])
                    drs = st.tile([128, 128], BF16, tag="drs", name="drs")
                    nc.vector.tensor_scalar_mul(drs[:pb, :pb],
                                                ident_bf[:pb, :pb],
                                                rsi[:pb])
                    for c in range(2):
                        p = psp.tile([128, 128], F32, tag="trp", bufs=2,
                                     name="ptw")
                        nc.tensor.matmul(p[:128, :pb],
                                         ewi[:pb, c * 128:(c + 1) * 128],
                                         drs[:pb, :pb],
                                         start=True, stop=True)
                        nc.vector.tensor_copy(iwT[:, c, off:off + pb],
                                              p[:128, :pb])
                r["iwT"] = iwT
                return r

            # ---------- feature-major GRU core ----------
            def gru_bn(w_gh_s, bhh_s, act_copies=False):
                """BN wave: gh_n x h + bhh_n. Depends only on hTs — emit as
                early as possible in the step."""
                bn_ps = []
                for ht in range(4):
                    sl = slice(2 * H + 128 * ht, 2 * H + 128 * (ht + 1))
                    ps = psp.tile([128, BS], F32, tag="gate", name="bn")
                    pairs = [(w_gh_s[:, kt, sl], hTs[kt][:, :])
                             for kt in range(4)]
                    pairs.append((bhh_s[0:1, 128 * ht:128 * (ht + 1)],
                                  ones_bf[0:1, :]))
                    emit_group(ps[:, :], pairs)
                    bn_ps.append(ps)
                bnF = [st.tile([128, BS], F32, tag=f"bnF{ht}", bufs=1,
                               name="bnF") for ht in range(4)]
                for ht in range(4):
                    (nc.scalar.copy if (act_copies or ht % 2) else
                     nc.vector.tensor_copy)(bnF[ht][:], bn_ps[ht][:, :])
                return bnF

            def gru_fm(w_gi_s, w_gh_s, bhh_s, xt, xrows, extra, bnF,
                       x_late=False):
                """extra: list of (sbuf_tile_or_list, kt_base). Updates
                hTs/hFs in place. Gate math pipelined per h-tile."""
                def xa(tile_sb, k):
                    if isinstance(tile_sb, list):
                        return tile_sb[k][:, 0:BS]
                    return tile_sb[:, k, 0:BS]

                def gate_wave(ci):
                    tiles = []
                    for ht in range(4):
                        sl = slice(ci * H + 128 * ht, ci * H + 128 * (ht + 1))
                        ps = psp.tile([128, BS], F32, tag="gate",
                                      name=f"g{ci}")
                        pairs = []
                        if ci < 2:
                            pairs += [(w_gh_s[:, kt, sl], hTs[kt][:, :])
                                      for kt in range(4)]
                        xpairs = [(w_gi_s[:nr, kt, sl], xt[0:nr, kt, 0:BS])
                                  for kt, nr in enumerate(xrows)]
                        if not x_late:
                            pairs += xpairs
                        for (tile_sb, ktb) in extra:
                            for k in range(4):
                                pairs.append((w_gi_s[:, ktb + k, sl],
                                              xa(tile_sb, k)))
                        if x_late:
                            pairs += xpairs
                        emit_group(ps[:, :], pairs)
                        tiles.append(ps)
                    return tiles

                r_ps = gate_wave(0)
                rF = [st.tile([128, BS], F32, tag=f"rF{ht}", bufs=1,
                              name="rF") for ht in range(4)]
                for ht in range(4):
                    nc.scalar.activation(rF[ht][:], r_ps[ht][:, :],
                                         AF.Sigmoid)
                z_ps = gate_wave(1)
                zF = [st.tile([128, BS], F32, tag=f"zF{ht}", bufs=1,
                              name="zF") for ht in range(4)]
                for ht in range(4):
                    nc.scalar.activation(zF[ht][:], z_ps[ht][:, :],
                                         AF.Sigmoid)
                n_ps = gate_wave(2)
                # per-tile chains: t1 = r*bn + n_ps; n = tanh(t1);
                # h' = n + z*(h-n); hT = bf16(h')
                for ht in range(4):
                    t1 = st.tile([128, BS], F32, tag=f"t1{ht}", bufs=1,
                                 name="t1")
                    nc.vector.tensor_mul(t1[:], rF[ht][:], bnF[ht][:])
                    nc.vector.tensor_add(t1[:], t1[:], n_ps[ht][:, :])
                    nF = st.tile([128, BS], F32, tag=f"nF{ht}", bufs=1,
                                 name="nF")
                    nc.scalar.activation(nF[:], t1[:], AF.Tanh)
                    dd = st.tile([128, BS], F32, tag=f"dd{ht}", bufs=1,
                                 name="dd")
                    eng = nc.gpsimd if ht % 2 else nc.vector
                    eng.tensor_sub(dd[:], hFs[ht][:], nF[:])
                    eng.tensor_mul(dd[:], dd[:], zF[ht][:])
                    eng.tensor_add(hFs[ht][:], dd[:], nF[:])
                    nc.vector.tensor_copy(hTs[ht][:], hFs[ht][:])

            # ---------- image projections ----------
            for mt in range(2):
                psv = psp.tile([128, H], F32, tag="gate", name="psv")
                emit_group(psv[:], [(s_imgt[:, kt, mt * 128:(mt + 1) * 128],
                                     s_iv[:, kt, :]) for kt in range(2)])
                nc.scalar.copy(ivv[:, mt, :], psv[:])
            for mt in range(2):
                psik = psp.tile([128, 128], F32, tag="gate", name="psik")
                emit_group(psik[:K, :],
                           [(s_ik[:, kt, :],
                             s_imgt[:, kt, mt * 128:(mt + 1) * 128])
                            for kt in range(2)])
                nc.vector.tensor_copy(ikt2[:K, mt, :], psik[:K, :])

            # ---------- encoder ----------
            with tc.tile_pool(name="qp", bufs=1) as qp:
                s_egi = load(qp, w_egi, BF16)
                s_egh = load(qp, w_egh, BF16)
                xt_q = fetch_x(s_qidx, 0)
                # decoder weights: prefetch now, overlapping encoder compute
                s_gi = load(pers, w_gi, BF16)
                s_gh = load(pers, w_gh, BF16)
                s_out = load(pers, w_out, BF16)
                for t in range(L):
                    bnF = gru_bn(s_egh, s_ebhh)
                    gru_fm(s_egi, s_egh, s_ebhh, xt_q, [128, 128, 65], [],
                           bnF)
                    if t + 1 < L:
                        xt_q = fetch_x(s_qidx, t + 1)
                    for bt in range(2):
                        pb, off = PBS[bt], BOFF[bt]
                        sl = slice(off, off + pb)
                        psk = psp.tile([128, K], F32, tag="gate", name="psk")
                        emit_group(psk[:pb, :],
                                   [(hTs[kt][:, sl], s_qk[:, kt, :])
                                    for kt in range(4)])
                        nc.scalar.copy(qkbs[bt][:pb, t, :], psk[:pb, :])
                        psv = psp.tile([128, H], F32, tag="gate", name="psv")
                        emit_group(psv[:pb, :],
                                   [(hTs[kt][:, sl], s_qv[:, kt, :])
                                    for kt in range(4)])
                        if bt == 0:
                            nc.scalar.copy(qv_b0[:pb, t, :], psv[:pb, :])
                        else:
                            g4 = t % 4
                            nc.scalar.copy(
                                qv_p1[32 * g4:32 * (g4 + 1), t // 4, :],
                                psv[:pb, :])

            for i in range(4):
                nc.vector.memset(hTs[i][:], 0.0)
                nc.vector.memset(hFs[i][:], 0.0)

            # ---------- decoder ----------
            with tc.tile_pool(name="lg", bufs=1) as lg:
                o19T = lg.tile([128, 3, BS], FP8)
                nc.vector.memset(o19T[32:64, 2, :], 0.0)
                nc.vector.memset(o19T[64:65, 2, :], 1.0)
                o19_0 = lg.tile([128, D], F32)
                o19_1 = lg.tile([128, D], F32)
                o19_sb = [o19_0, o19_1]

                xt_a = fetch_x(s_aidx, 0)
                at = attn_phase(True)   # for step 0
                for t in range(MAX_LEN):
                    dg2, dg1b = at["dg"]
                    iwT = at["iwT"]

                    bnF = gru_bn(s_gh, s_bhh, act_copies=(t == L))

                    # --- icT (feature-major) ---
                    icT = [wk.tile([128, BS], BF16, tag=f"icT{k}", bufs=2,
                                   name="icT") for k in range(4)]
                    for ht in range(4):
                        ps = psp.tile([128, BS], F32, tag="gate", name="psic")
                        emit_group(ps[:, :],
                                   [(ivv[:, mt, 128 * ht:128 * (ht + 1)],
                                     iwT[:, mt, :]) for mt in range(2)])
                        nc.scalar.activation(icT[ht][:], ps[:, :],
                                             AF.Identity,
                                             bias=s_ivb[:, ht:ht + 1],
                                             scale=1.0)

                    # --- qcT (feature-major, symmetric diag trick) ---
                    qcT = [wk.tile([128, BS], BF16, tag=f"qcT{k}", bufs=2,
                                   name="qcT") for k in range(4)]
                    for ht in range(4):
                        hsl = slice(128 * ht, 128 * (ht + 1))
                        ps = psp.tile([128, BS], F32, tag="gate", name="psqc")
                        first = True
                        for l in range(L):
                            nc.tensor.matmul(ps[:, 0:128],
                                             qv_b0[:, l, hsl],
                                             dg2[:, :, l:l + 1],
                                             start=first, stop=False)
                            first = False
                        for c in range(5):
                            nc.tensor.matmul(ps[:, 128:BS],
                                             qv_p1[:, c, hsl],
                                             dg1b[:, :, c:c + 1],
                                             start=False, stop=(c == 4))
                        if ht % 2 or t == L:
                            nc.scalar.activation(qcT[ht][:], ps[:, :],
                                                 AF.Identity,
                                                 bias=s_qvb[:, ht:ht + 1],
                                                 scale=1.0)
                        else:
                            nc.vector.tensor_scalar_add(qcT[ht][:], ps[:, :],
                                                        s_qvb[:, ht:ht + 1])

                    # --- GRU ---
                    xsrc = xt_a if t < L else dec20
                    gru_fm(s_gi, s_gh, s_bhh, xsrc, [128, 128, 65],
                           [(icT, 7), (qcT, 3)], bnF, x_late=(t == L))
                    if t < L - 1:
                        xt_a = fetch_x(s_aidx, t + 1)

                    # --- attention phase for next step (overlaps out-proj) ---
                    if t + 1 < MAX_LEN:
                        at = attn_phase(True)

                    # --- output projection ---
                    if t != MAX_LEN - 2:
                        osbT = st.tile([128, 3, BS], F32, tag="osbT",
                                       name="osbT")
                        nc.vector.memset(osbT[32:64, 2, :], 0.0)
                        nc.vector.memset(osbT[64:, 2, :], 0.0)
                        for dt_ in range(3):
                            nd = 128 if dt_ < 2 else D - 256
                            sl = slice(128 * dt_, 128 * dt_ + nd)
                            ps = psp.tile([128, BS], F32, tag="gate",
                                          name="pso")
                            pairs = [(s_out[:, 4 + k, sl], qcT[k][:, 0:BS])
                                     for k in range(4)]
                            pairs += [(s_out[:, 8 + k, sl], icT[k][:, 0:BS])
                                      for k in range(4)]
                            pairs.append((s_outb[0:1, sl], ones_bf[0:1, :]))
                            pairs += [(s_out[:, k, sl], hTs[k][:, :])
                                      for k in range(4)]
                            n = len(pairs)
                            for i, (lh, rh) in enumerate(pairs):
                                nc.tensor.matmul(ps[:nd, :], lh, rh,
                                                 start=(i == 0),
                                                 stop=(i == n - 1))
                            nc.vector.tensor_copy(osbT[:nd, dt_, :],
                                                  ps[:nd, :])
                        nc.sync.dma_start(out_o[t], osbT[:])
                    else:
                        # t == 19: batch-major out for argmax rescoring
                        for bt in range(2):
                            pb, off = PBS[bt], BOFF[bt]
                            sl = slice(off, off + pb)
                            pso = psp.tile([128, D], F32, tag="gate",
                                           name="pso19")
                            pairs = [(hTs[k][:, sl], s_out[:, k, :])
                                     for k in range(4)]
                            pairs += [(qcT[k][:, sl], s_out[:, 4 + k, :])
                                      for k in range(4)]
                            pairs += [(icT[k][:, sl], s_out[:, 8 + k, :])
                                      for k in range(4)]
                            pairs.append((ones_bf[0:1, :pb], s_outb[:]))
                            emit_group(pso[:pb, :], pairs)
                            osb = o19_sb[bt]
                            nc.scalar.copy(osb[:pb], pso[:pb])
                            nc.sync.dma_start(out_o19[:pb, bt, :], osb[:pb])
                            ob = st.tile([128, D], BF16, tag="ob",
                                         name="ob")
                            nc.scalar.copy(ob[:pb], osb[:pb])
                            for kt in range(3):
                                w = 128 if kt < 2 else D - 256
                                p8t = psp.tile([128, 128], BF16, tag="trp",
                                               bufs=2, name="p8t")
                                nc.tensor.transpose(p8t[:w, :pb],
                                                    ob[:pb,
                                                       kt * 128:kt * 128 + w],
                                                    ident_bf[:pb, :pb])
                                nc.vector.tensor_copy(
                                    o19T[:w, kt, off:off + pb], p8t[:w, :pb])

                    # --- argmax + re-embed for step 20 ---
                    if t == MAX_LEN - 2:
                        mx18 = [lg.tile([128, 144], F32, name="mx18a"),
                                lg.tile([128, 144], F32, name="mx18b")]
                        ix18 = [lg.tile([128, 144], F32, name="ix18a"),
                                lg.tile([128, 144], F32, name="ix18b")]
                        for nci in range(18):
                            ncw = 512 if nci < 17 else V - 17 * 512
                            rhs = wk.tile([128, 3, 512], FP8, tag="lrhs",
                                          bufs=4, name="rhs")
                            for kt in range(3):
                                nr = 128 if kt < 2 else 65
                                nc.sync.dma_start(
                                    rhs[:nr, kt, :ncw],
                                    embt_bf[:nr, kt,
                                            nci * 512:nci * 512 + ncw])
                            for bt in range(2):
                                pb, off = PBS[bt], BOFF[bt]
                                psl = psp.tile([128, H], F32, tag="gate",
                                               name="psl")
                                if ncw < 512:
                                    nc.vector.memset(psl[:pb, ncw:], NEG)
                                nc.tensor.matmul(
                                    psl[:pb, :ncw],
                                    o19T[:, 0:2, off:off + pb],
                                    rhs[:, 0:2, :ncw],
                                    start=True, stop=False,
                                    perf_mode=PM.DoubleRow)
                                nc.tensor.matmul(
                                    psl[:pb, :ncw],
                                    o19T[:65, 2, off:off + pb],
                                    rhs[:65, 2, :ncw],
                                    start=False, stop=True)
                                nc.vector.max(
                                    mx18[bt][:pb, 8 * nci:8 * nci + 8],
                                    psl[:pb, :])
                                ixc = st.tile([128, 8], U32, tag="ixc",
                                              bufs=3, name="ixc")
                                nc.vector.max_index(
                                    ixc[:pb],
                                    mx18[bt][:pb, 8 * nci:8 * nci + 8],
                                    psl[:pb, :])
                                nc.vector.tensor_copy(
                                    ix18[bt][:pb, 8 * nci:8 * nci + 8],
                                    ixc[:pb])
                        g8s = {}
                        for bt in range(2):
                            pb, off = PBS[bt], BOFF[bt]
                            ixg = st.tile([128, 144], F32, tag=f"ixg{bt}",
                                          name="ixg")
                            nc.vector.tensor_add(ixg[:pb], ix18[bt][:pb],
                                                 off18[:pb])
                            c8 = st.tile([128, 8], F32, tag=f"c8{bt}",
                                         name="c8")
                            nc.vector.max(c8[:pb], mx18[bt][:pb])
                            p8 = st.tile([128, 8], U32, tag=f"p8{bt}",
                                         name="p8")
                            nc.vector.max_index(p8[:pb], c8[:pb],
                                                mx18[bt][:pb])
                            p8f = st.tile([128, 8], F32, tag=f"p8f{bt}",
                                          name="p8f")
                            nc.vector.tensor_copy(p8f[:pb], p8[:pb])
                            ix8f = st.tile([128, 8], F32, tag=f"ix8f{bt}",
                                           name="ix8f")
                            g8s[bt] = []
                            for j in range(8):
                                oh = wk.tile([128, 144], F32, tag="oh144",
                                             bufs=2, name="oh")
                                nc.vector.tensor_scalar(
                                    out=oh[:pb], in0=iota144[:pb],
                                    scalar1=p8f[:pb, j:j + 1], scalar2=None,
                                    op0=ALU.is_equal)
                                nc.vector.tensor_mul(oh[:pb], oh[:pb],
                                                     ixg[:pb])
                                vj = st.tile([128, 1], F32, tag="vj",
                                             name="vj")
                                nc.vector.tensor_reduce(vj[:pb], oh[:pb],
                                                        axis=AX.X, op=ALU.add)
                                nc.vector.tensor_copy(ix8f[:pb, j:j + 1],
                                                      vj[:pb])
                                vju = st.tile([128, 1], U32, tag="vju",
                                              name="vju")
                                nc.vector.tensor_copy(vju[:pb], vj[:pb])
                                g8 = wk.tile([128, D + 1], F32, tag="gath8",
                                             bufs=10, name="g8")
                                nc.gpsimd.indirect_dma_start(
                                    out=g8[:pb], out_offset=None,
                                    in_=emb_aug[:],
                                    in_offset=bass.IndirectOffsetOnAxis(
                                        ap=vju[:pb, 0:1], axis=0))
                                g8s[bt].append(g8)
                            g8s[bt + 2] = ix8f
                        for bt in range(2):
                            pb, off = PBS[bt], BOFF[bt]
                            ix8f = g8s[bt + 2]
                            scores = st.tile([128, 8], F32, tag=f"sc8{bt}",
                                             name="scores")
                            for j in range(8):
                                g8 = g8s[bt][j]
                                pr = wk.tile([128, D], F32, tag="pr8",
                                             name="pr")
                                nc.vector.tensor_mul(pr[:pb], o19_sb[bt][:pb],
                                                     g8[:pb, :D])
                                sj = st.tile([128, 1], F32, tag="sj",
                                             name="sj")
                                nc.vector.tensor_reduce(sj[:pb], pr[:pb],
                                                        axis=AX.X, op=ALU.add)
                                nc.vector.tensor_add(scores[:pb, j:j + 1],
                                                     sj[:pb],
                                                     g8[:pb, D:D + 1])
                            m1 = st.tile([128, 8], F32, tag="m1", name="m1")
                            nc.vector.max(m1[:pb], scores[:pb])
                            j1 = st.tile([128, 8], U32, tag="j1", name="j1")
                            nc.vector.max_index(j1[:pb], m1[:pb],
                                                scores[:pb])
                            j1f = st.tile([128, 1], F32, tag="j1f",
                                          name="j1f")
                            nc.vector.tensor_copy(j1f[:pb], j1[:pb, 0:1])
                            oh8 = st.tile([128, 8], F32, tag="oh8",
                                          name="oh8")
                            nc.vector.tensor_scalar(out=oh8[:pb],
                                                    in0=iota8[:pb],
                                                    scalar1=j1f[:pb],
                                                    scalar2=None,
                                                    op0=ALU.is_equal)
                            nc.vector.tensor_mul(ix8f[:pb], oh8[:pb],
                                                 ix8f[:pb])
                            vsum = st.tile([128, 1], F32, tag="vsum",
                                           name="vsum")
                            nc.vector.tensor_reduce(vsum[:pb], ix8f[:pb],
                                                    axis=AX.X, op=ALU.add)
                            vidx = st.tile([128, 1], U32, tag="vidx",
                                           name="vidx")
                            nc.vector.tensor_copy(vidx[:pb], vsum[:pb])
                            gm = wk.tile([128, D], BF16, tag="gath", bufs=6,
                                         name="gm")
                            nc.gpsimd.indirect_dma_start(
                                out=gm[:pb], out_offset=None, in_=emb_bf[:],
                                in_offset=bass.IndirectOffsetOnAxis(
                                    ap=vidx[:pb, 0:1], axis=0))
                            for kt in range(3):
                                w = 128 if kt < 2 else D - 256
                                tr(dec20[:w, kt, off:off + pb],
                                   gm[:pb, kt * 128:kt * 128 + w], pb, w)

    nc.compile()
    return nc


_NC_CACHE = None


def _get_nc():
    global _NC_CACHE
    if _NC_CACHE is None:
        _NC_CACHE = build_nc()
    return _NC_CACHE


def _pad_tiles(a, ntiles):
    rows, cols = a.shape
    out = np.zeros((128 * ntiles, cols), a.dtype)
    out[:rows] = a
    return np.ascontiguousarray(
        out.reshape(ntiles, 128, cols).transpose(1, 0, 2))


def _prep_shared(inputs):
    bf = np.float16
    f32 = np.float32
    eW = np.asarray(inputs["embed_W"], f32)
    d = {}
    wih = np.asarray(inputs["dec_W_ih"], f32)
    bih = np.asarray(inputs["dec_b_ih"], f32)
    bhh = np.asarray(inputs["dec_b_hh"], f32)
    gi = np.zeros((128 * 11, 3 * H), f32)
    gi[0:D] = wih[:, 0:D].T
    gi[320] = bih + np.concatenate([bhh[:2 * H], np.zeros(H, f32)])
    gi[384:384 + H] = wih[:, D:D + H].T
    gi[896:896 + H] = wih[:, D + H:].T
    d["w_gi"] = _pad_tiles(gi.astype(bf), 11)
    d["w_gh"] = _pad_tiles(np.asarray(inputs["dec_W_hh"], f32).T.astype(bf), 4)
    d["bhh_n"] = np.ascontiguousarray(bhh[2 * H:].astype(bf)[None, :])
    ewih = np.asarray(inputs["enc_W_ih"], f32)
    ebih = np.asarray(inputs["enc_b_ih"], f32)
    ebhh = np.asarray(inputs["enc_b_hh"], f32)
    egi = np.zeros((128 * 3, 3 * H), f32)
    egi[0:D] = ewih[:, :D].T
    egi[320] = ebih + np.concatenate([ebhh[:2 * H], np.zeros(H, f32)])
    d["w_egi"] = _pad_tiles(egi.astype(bf), 3)
    d["w_egh"] = _pad_tiles(np.asarray(inputs["enc_W_hh"], f32).T.astype(bf), 4)
    d["ebhh_n"] = np.ascontiguousarray(ebhh[2 * H:].astype(bf)[None, :])
    d["w_out"] = _pad_tiles(np.asarray(inputs["out_W"], f32).T.astype(bf), 12)
    d["outb"] = np.ascontiguousarray(
        np.asarray(inputs["out_b"], f32).astype(bf)[None, :])
    d["w_qk"] = _pad_tiles(np.asarray(inputs["qk_W"], f32).T.astype(bf), 4)
    d["w_qv"] = _pad_tiles(np.asarray(inputs["qv_W"], f32).T.astype(bf), 4)
    d["qvb_c"] = np.ascontiguousarray(
        np.asarray(inputs["qv_b"], f32).reshape(4, 128).T)
    d["w_ak"] = _pad_tiles(np.asarray(inputs["ak_W"], f32).T.astype(bf), 4)
    d["akb"] = np.ascontiguousarray(
        np.asarray(inputs["ak_b"], f32).astype(bf)[None, :])
    d["w_ik"] = _pad_tiles(np.asarray(inputs["ik_W"], f32).T.astype(bf), 2)
    d["w_iv"] = _pad_tiles(np.asarray(inputs["iv_W"], f32).T.astype(bf), 2)
    d["ivb_c"] = np.ascontiguousarray(
        np.asarray(inputs["iv_b"], f32).reshape(4, 128).T)
    d["emb_bf"] = eW.astype(bf)
    wd_b = np.asarray(inputs["wd_b"], f32)
    d["emb_aug"] = np.ascontiguousarray(
        np.concatenate([eW, wd_b[:, None]], 1))
    aug = np.zeros((128 * 3, VP), f32)
    aug[:D, :V] = eW.T
    aug[320, :V] = wd_b
    import ml_dtypes
    d["embt_bf"] = _pad_tiles(aug.astype(ml_dtypes.float8_e4m3fn), 3)
    return d


def _idx_cols(seq_rows):
    out = np.zeros((128, 2 * L), np.uint32)
    for t in range(L):
        out[:, 2 * t] = seq_rows[0:128, t]
        out[:32, 2 * t + 1] = seq_rows[128:160, t]
    return out


def _build_maps(inputs, shared):
    f32 = np.float32
    bf = np.float16
    ques = np.asarray(inputs["ques_seqs"]).astype(np.uint32)
    ans = np.asarray(inputs["ans_seqs"]).astype(np.uint32)
    qlens = np.asarray(inputs["ques_lens"]).astype(np.int64)
    img = np.asarray(inputs["img_seqs"], f32)
    maps = []
    for s in range(NCORES):
        m = dict(shared)
        r0 = s * BS
        m["q_idx"] = _idx_cols(ques[r0:r0 + BS, :L])
        m["a_idx"] = _idx_cols(ans[r0:r0 + BS, :L])
        qm = np.full((128, 2, L), NEG, f32)
        lens = qlens[r0:r0 + BS]
        for bt, (pb, off) in enumerate(zip(PBS, BOFF)):
            for b in range(pb):
                qm[b, bt, :lens[off + b]] = 0.0
        m["qe_mask"] = qm.astype(bf)
        im = np.full((128, 2, 256), NEG, f32)
        for bt, (pb, off) in enumerate(zip(PBS, BOFF)):
            for b in range(pb):
                gimg = (off + b) // ROUNDS
                im[b, bt, gimg * 16:(gimg + 1) * 16] = 0.0
        m["ie_mask"] = im.astype(bf)
        imgs = img[s * 16:(s + 1) * 16].reshape(256, 256)
        it = np.zeros((128 * 2, 256), f32)
        it[:256] = imgs.T
        m["img_t"] = np.ascontiguousarray(
            it.reshape(2, 128, 256).transpose(1, 0, 2)).astype(bf)
        maps.append(m)
    return maps


def kernel(**inputs):
    nc = _get_nc()
    shared = _prep_shared(inputs)
    in_maps = _build_maps(inputs, shared)
    from concourse.bass_utils import run_bass_kernel_spmd
    res = run_bass_kernel_spmd(nc, in_maps, core_ids=list(range(NCORES)))
    outs = []
    for s in range(NCORES):
        o = np.asarray(res.results[s]["out_o"])      # [21, 128, 3, 160]
        o19 = np.asarray(res.results[s]["out_o19"])  # [128, 2, 300]
        full = o.transpose(3, 0, 2, 1).reshape(BS, MAX_LEN, 384)[:, :, :D]
        full = np.ascontiguousarray(full)
        full[:128, MAX_LEN - 2] = o19[:, 0, :]
        full[128:, MAX_LEN - 2] = o19[:32, 1, :]
        outs.append(full)
    return np.concatenate(outs, 0).astype(np.float32)


# revision 5
# speedup vs baseline: 1.0538x; 1.0164x over previous
"""Trainium2 Bass kernel for nn_BaselineAttnDecoder — feature-major,
software-pipelined.

Per core: 160 decode rows (16 images x 10 rounds), weights replicated.

- GRU gates / out-proj / icT / qcT all FEATURE-major: out [feat<=128, 160]
  PSUM groups, moving dim = true batch 160 (no 128+32 padding waste).
- h lives only as per-tile hT (bf16) + hF (f32): no h transposes.
- qcT via symmetric diag-trick (lhsT = batch-major q_value, rhs = the
  qw diagonal) — attention-weighted sum lands directly feature-major.
- Attention score chain (a, qe, iw softmax, diag build) for step t+1 is
  emitted in step t's tail so the DVE chain overlaps PE matmul work.
- Gate elementwise math runs per h-tile, pipelined across Act/DVE/Pool.
- Step-19 vocab argmax: blockwise top-8 from PSUM chunks (bf16 copies on
  Act, Max/MaxIndex on DVE), exact-f32 rescore of 8 candidates.
"""
import numpy as np

import concourse.bass as bass
import concourse.bacc as bacc
import concourse.mybir as mybir
import concourse.tile as tile
from concourse.masks import make_identity

F32 = mybir.dt.float32
BF16 = mybir.dt.float16  # 16-bit compute dtype (f16: 10-bit mantissa)
U32 = mybir.dt.uint32
FP8 = mybir.dt.float8e4
PM = mybir.MatmulPerfMode
AF = mybir.ActivationFunctionType
ALU = mybir.AluOpType
AX = mybir.AxisListType

D, H, V, K = 300, 512, 8835, 50
L, MAX_LEN, ROUNDS = 20, 21, 10
BS = 160
NCORES = 8
PBS = [128, 32]
BOFF = [0, 128]
VP = 18 * 512
NEG = -60000.0


def bcast_mid(ap, reps):
    return bass.AP(tensor=ap.tensor, offset=ap.offset,
                   ap=[ap.ap[0], [0, reps], ap.ap[1]])


def bcast_in(ap, reps):
    return bass.AP(tensor=ap.tensor, offset=ap.offset,
                   ap=[ap.ap[0], ap.ap[1], [0, reps]])


def build_nc():
    nc = bacc.Bacc()

    def din(name, shape, dt):
        return nc.dram_tensor(name, shape, dt, kind="ExternalInput")

    w_gi = din("w_gi", [128, 11, 3 * H], BF16)
    w_gh = din("w_gh", [128, 4, 3 * H], BF16)
    bhh_n = din("bhh_n", [1, H], BF16)
    w_egi = din("w_egi", [128, 3, 3 * H], BF16)
    w_egh = din("w_egh", [128, 4, 3 * H], BF16)
    ebhh_n = din("ebhh_n", [1, H], BF16)
    w_out = din("w_out", [128, 12, D], BF16)
    outb = din("outb", [1, D], BF16)
    w_qk = din("w_qk", [128, 4, K], BF16)
    w_qv = din("w_qv", [128, 4, H], BF16)
    w_ak = din("w_ak", [128, 4, K], BF16)
    akb = din("akb", [1, K], BF16)
    w_ik = din("w_ik", [128, 2, K], BF16)
    w_iv = din("w_iv", [128, 2, H], BF16)
    ivb_c = din("ivb_c", [128, 4], F32)
    qvb_c = din("qvb_c", [128, 4], F32)
    img_t = din("img_t", [128, 2, 2 * 128], BF16)
    emb_bf = din("emb_bf", [V, D], BF16)
    emb_aug = din("emb_aug", [V, D + 1], F32)
    embt_bf = din("embt_bf", [128, 3, VP], FP8)
    q_idx = din("q_idx", [128, 2 * L], U32)
    a_idx = din("a_idx", [128, 2 * L], U32)
    qe_mask = din("qe_mask", [128, 2, L], BF16)
    ie_mask = din("ie_mask", [128, 2, 2 * 128], BF16)

    out_o = nc.dram_tensor("out_o", [MAX_LEN, 128, 3, BS], F32,
                           kind="ExternalOutput")
    out_o19 = nc.dram_tensor("out_o19", [128, 2, D], F32,
                             kind="ExternalOutput")

    with tile.TileContext(nc) as tc:
        with (
            tc.tile_pool(name="cw", bufs=1) as cw,
            tc.tile_pool(name="pers", bufs=1) as pers,
            tc.tile_pool(name="wk", bufs=2) as wk,
            tc.tile_pool(name="st", bufs=2) as st,
            tc.tile_pool(name="ps", bufs=6, space="PSUM") as psp,
        ):
            def load(pool, t, dt):
                s = pool.tile(list(t.shape), dt, name=t.name + "_sb")
                nc.sync.dma_start(s[:], t[:])
                return s

            # encoder-critical loads first (SP queue is in-order)
            s_qidx = load(cw, q_idx, U32)
            s_egi_w = load(cw, w_egi, BF16)
            s_egh_w = load(cw, w_egh, BF16)
            s_ebhh = load(cw, ebhh_n, BF16)
            s_qk = load(cw, w_qk, BF16)
            s_qv = load(cw, w_qv, BF16)
            s_ak = load(cw, w_ak, BF16)
            s_ik = load(cw, w_ik, BF16)
            s_iv = load(cw, w_iv, BF16)
            s_imgt = load(cw, img_t, BF16)
            s_bhh = load(cw, bhh_n, BF16)
            s_outb = load(cw, outb, BF16)
            s_akb = load(cw, akb, BF16)
            s_ivb = load(cw, ivb_c, F32)
            s_qvb = load(cw, qvb_c, F32)
            s_aidx = load(cw, a_idx, U32)
            s_qem = load(cw, qe_mask, BF16)
            s_iem = load(cw, ie_mask, BF16)

            ident_bf = cw.tile([128, 128], BF16)
            make_identity(nc, ident_bf[:])
            ones_bf = cw.tile([1, BS], BF16)
            nc.vector.memset(ones_bf[:], 1.0)
            sid4 = cw.tile([128, 32], BF16)
            for g4 in range(4):
                nc.vector.tensor_copy(sid4[32 * g4:32 * (g4 + 1), :],
                                      ident_bf[0:32, 0:32])
            iota8 = cw.tile([128, 8], F32)
            nc.gpsimd.iota(iota8[:], pattern=[[1, 8]], base=0,
                           channel_multiplier=0,
                           allow_small_or_imprecise_dtypes=True)
            iota144 = cw.tile([128, 144], F32)
            nc.gpsimd.iota(iota144[:], pattern=[[1, 144]], base=0,
                           channel_multiplier=0,
                           allow_small_or_imprecise_dtypes=True)
            off18 = cw.tile([128, 144], F32)
            nc.gpsimd.iota(off18[:], pattern=[[512, 18], [0, 8]], base=0,
                           channel_multiplier=0,
                           allow_small_or_imprecise_dtypes=True)
            # identity replicated along an inner L/5 axis (for diag builds
            # that keep innermost stride-1 so DVE 2x mode applies)
            i_rep = cw.tile([128, 128, L], BF16)
            for l in range(L):
                nc.vector.tensor_copy(i_rep[:, :, l:l + 1],
                                      bass.AP(tensor=ident_bf.tensor,
                                              offset=ident_bf[:, :].offset,
                                              ap=[ident_bf[:, :].ap[0],
                                                  [1, 128], [0, 1]]))
            sid_rep = cw.tile([128, 32, 5], BF16)
            for c in range(5):
                nc.vector.tensor_copy(sid_rep[:, :, c:c + 1],
                                      bass.AP(tensor=sid4.tensor,
                                              offset=sid4[:, :].offset,
                                              ap=[sid4[:, :].ap[0],
                                                  [1, 32], [0, 1]]))

            # persistent state — h per tile, feature-major
            hTs = [pers.tile([128, BS], BF16, name=f"hT{i}") for i in range(4)]
            hFs = [pers.tile([128, BS], F32, name=f"hF{i}") for i in range(4)]
            qk_b0 = pers.tile([128, L, K], BF16)
            qk_b1 = pers.tile([128, L, K], BF16)
            qkbs = [qk_b0, qk_b1]
            qv_b0 = pers.tile([128, L, H], BF16)
            qv_p1 = pers.tile([128, 5, H], BF16)
            ivv = pers.tile([128, 2, H], BF16)
            ikt2 = pers.tile([128, 2, 128], BF16)
            dec20 = pers.tile([128, 3, BS], BF16)

            for i in range(4):
                nc.vector.memset(hTs[i][:], 0.0)
                nc.vector.memset(hFs[i][:], 0.0)
            nc.vector.memset(dec20[32:64, 2, :], 0.0)
            nc.vector.memset(dec20[64:65, 2, :], 1.0)

            def tr(dst_sb_ap, src_sb_ap, pb, w, eng=None):
                p = psp.tile([128, 128], BF16, tag="trp", bufs=2, name="pt")
                nc.tensor.transpose(p[:w, :pb], src_sb_ap, ident_bf[:pb, :pb])
                (eng or nc.vector).tensor_copy(dst_sb_ap, p[:w, :pb])

            def fetch_x(idx_sb, t):
                xt = wk.tile([128, 3, BS], BF16, tag="xt", bufs=3, name="xt")
                nc.vector.memset(xt[32:64, 2, :], 0.0)
                nc.vector.memset(xt[64:65, 2, :], 1.0)
                for c, (pb, off) in enumerate(zip(PBS, BOFF)):
                    g = wk.tile([128, D], BF16, tag="gath", bufs=6, name="g")
                    nc.gpsimd.indirect_dma_start(
                        out=g[:pb], out_offset=None, in_=emb_bf[:],
                        in_offset=bass.IndirectOffsetOnAxis(
                            ap=idx_sb[:pb, 2 * t + c:2 * t + c + 1], axis=0))
                    for kt in range(3):
                        w = 128 if kt < 2 else D - 256
                        if kt == 1:
                            p = psp.tile([128, 128], BF16, tag="trp", bufs=2,
                                         name="pt")
                            nc.tensor.transpose(p[:w, :pb],
                                                g[:pb, 128:128 + w],
                                                ident_bf[:pb, :pb])
                            nc.scalar.copy(xt[:w, kt, off:off + pb],
                                           p[:w, :pb])
                        else:
                            tr(xt[:w, kt, off:off + pb],
                               g[:pb, kt * 128:kt * 128 + w], pb, w)
                return xt

            def emit_group(ps_ap, pairs):
                n = len(pairs)
                for i, (lh, rh) in enumerate(pairs):
                    nc.tensor.matmul(ps_ap, lh, rh, start=(i == 0),
                                     stop=(i == n - 1))

            # ---------- attention-score phase for step t (emitted in the
            # tail of step t-1; depends only on hTs) ----------
            def attn_phase(dec):
                """Returns dict with qw diag tiles + iwT for the next step."""
                r = {}
                # a = ak(h) + akb (batch-major), aT
                a_bf = st.tile([128, 2, K], BF16, tag="a_bf", name="a_bf")
                aT = st.tile([128, BS], BF16, tag="aT", name="aT")
                for bt in range(2):
                    pb, off = PBS[bt], BOFF[bt]
                    sl = slice(off, off + pb)
                    psa = psp.tile([128, K], F32, tag="gate", name="psa")
                    pairs = [(hTs[kt][:, sl], s_ak[:, kt, :])
                             for kt in range(4)]
                    pairs.append((ones_bf[0:1, :pb], s_akb[:]))
                    emit_group(psa[:pb, :], pairs)
                    nc.scalar.copy(a_bf[:pb, bt, :], psa[:pb, :])
                    tr(aT[:K, off:off + pb], a_bf[:pb, bt, :], pb, K)
                if not dec:
                    return r
                # question attention softmax -> normalized diag tiles.
                # dg2[b', b, l] = ew[b', l] * rs[b'] * I[b', b] in ONE
                # scalar_tensor_tensor (2x mode: all innermost stride-1).
                dg2 = wk.tile([128, 128, L], BF16, tag="dg2", bufs=2,
                              name="dg2")
                dg1b = wk.tile([128, 32, 5], BF16, tag="dg1b", bufs=2,
                              name="dg1b")
                ews = []
                rss = []
                for bt in range(2):
                    pb = PBS[bt]
                    prod = wk.tile([128, L, K], BF16, tag="prod", bufs=2,
                                   name="prod")
                    nc.vector.tensor_mul(prod[:pb], qkbs[bt][:pb],
                                         bcast_mid(a_bf[:pb, bt, :], L))
                    qe = st.tile([128, L], BF16, tag="qe", name="qe")
                    with nc.allow_low_precision(reason="attn scores bf16"):
                        nc.vector.tensor_reduce(qe[:pb], prod[:pb],
                                                axis=AX.X, op=ALU.add)
                    nc.vector.tensor_add(qe[:pb], qe[:pb], s_qem[:pb, bt, :])
                    nm = st.tile([128, 1], F32, tag="nm", name="nm")
                    nc.vector.tensor_reduce(nm[:pb], qe[:pb], axis=AX.X,
                                            op=ALU.max, negate=True)
                    ew = st.tile([128, L], BF16, tag="ew", name="ew")
                    ssum = st.tile([128, 1], F32, tag="ssum", name="ssum")
                    nc.scalar.activation(ew[:pb], qe[:pb], AF.Exp,
                                         bias=nm[:pb], scale=1.0,
                                         accum_out=ssum[:pb])
                    rs = st.tile([128, 1], F32, tag="rs", name="rs")
                    nc.vector.reciprocal(rs[:pb], ssum[:pb])
                    ews.append(ew)
                    rss.append(rs)
                qwn = st.tile([128, L], BF16, tag="qwn", name="qwn")
                nc.vector.tensor_scalar_mul(qwn[:128, :], ews[0][:128, :],
                                            rss[0][:128, :])
                nc.vector.tensor_mul(dg2[:, :, :],
                                     bcast_mid(qwn[:128, :], 128),
                                     i_rep[:, :, :])
                ew_pk = st.tile([128, 5], BF16, tag="ew_pk", name="ew_pk")
                for g4 in range(4):
                    nc.vector.tensor_scalar_mul(
                        ew_pk[32 * g4:32 * (g4 + 1), :],
                        ews[1][0:32, g4:L:4], rss[1][0:32, :])
                nc.vector.tensor_mul(dg1b[:, :, :],
                                     bcast_mid(ew_pk[:, :], 32),
                                     sid_rep[:, :, :])
                r["dg"] = (dg2, dg1b)
                # image attention softmax -> iwT
                iwT = st.tile([128, 2, BS], BF16, tag="iwT", name="iwT")
                for bt in range(2):
                    pb, off = PBS[bt], BOFF[bt]
                    psi = psp.tile([128, 256], F32, tag="gate", name="psi")
                    nc.tensor.matmul(psi[:pb, :], aT[:K, off:off + pb],
                                     ikt2[:K, :, :], start=True, stop=True)
                    iem = st.tile([128, 256], BF16, tag="iem", name="iem")
                    with nc.allow_low_precision(reason="attn scores bf16"):
                        nc.vector.tensor_add(iem[:pb], psi[:pb],
                                             s_iem[:pb, bt, :])
                    nmi = st.tile([128, 1], F32, tag="nmi", name="nmi")
                    nc.vector.tensor_reduce(nmi[:pb], iem[:pb], axis=AX.X,
                                            op=ALU.max, negate=True)
                    ewi = st.tile([128, 256], BF16, tag="ewi", name="ewi")
                    ssi = st.tile([128, 1], F32, tag="ssi", name="ssi")
                    nc.scalar.activation(ewi[:pb], iem[:pb], AF.Exp,
                                         bias=nmi[:pb], scale=1.0,
                                         accum_out=ssi[:pb])
                    rsi = st.tile([128, 1], F32, tag="rsi", name="rsi")
                    nc.vector.reciprocal(rsi[:pb], ssi[:pb])
                    drs = st.tile([128, 128], BF16, tag="drs", name="drs")
                    nc.vector.tensor_scalar_mul(drs[:pb, :pb],
                                                ident_bf[:pb, :pb],
                                                rsi[:pb])
                    for c in range(2):
                        p = psp.tile([128, 128], F32, tag="trp", bufs=2,
                                     name="ptw")
                        nc.tensor.matmul(p[:128, :pb],
                                         ewi[:pb, c * 128:(c + 1) * 128],
                                         drs[:pb, :pb],
                                         start=True, stop=True)
                        nc.vector.tensor_copy(iwT[:, c, off:off + pb],
                                              p[:128, :pb])
                r["iwT"] = iwT
                return r

            # ---------- feature-major GRU core ----------
            def gru_bn(w_gh_s, bhh_s, act_copies=False):
                """BN wave: gh_n x h + bhh_n. Depends only on hTs — emit as
                early as possible in the step."""
                bn_ps = []
                for ht in range(4):
                    sl = slice(2 * H + 128 * ht, 2 * H + 128 * (ht + 1))
                    ps = psp.tile([128, BS], F32, tag="gate", name="bn")
                    pairs = [(w_gh_s[:, kt, sl], hTs[kt][:, :])
                             for kt in range(4)]
                    pairs.append((bhh_s[0:1, 128 * ht:128 * (ht + 1)],
                                  ones_bf[0:1, :]))
                    emit_group(ps[:, :], pairs)
                    bn_ps.append(ps)
                bnF = [st.tile([128, BS], F32, tag=f"bnF{ht}", bufs=1,
                               name="bnF") for ht in range(4)]
                for ht in range(4):
                    (nc.scalar.copy if (act_copies or ht % 2) else
                     nc.vector.tensor_copy)(bnF[ht][:], bn_ps[ht][:, :])
                return bnF

            def gru_fm(w_gi_s, w_gh_s, bhh_s, xt, xrows, extra, bnF,
                       x_late=False, fill_fn=None):
                """extra: list of (sbuf_tile_or_list, kt_base). Updates
                hTs/hFs in place. Gate math pipelined per h-tile."""
                def xa(tile_sb, k):
                    if isinstance(tile_sb, list):
                        return tile_sb[k][:, 0:BS]
                    return tile_sb[:, k, 0:BS]

                def gate_wave(ci):
                    tiles = []
                    for ht in range(4):
                        sl = slice(ci * H + 128 * ht, ci * H + 128 * (ht + 1))
                        ps = psp.tile([128, BS], F32, tag="gate",
                                      name=f"g{ci}")
                        pairs = []
                        if ci < 2:
                            pairs += [(w_gh_s[:, kt, sl], hTs[kt][:, :])
                                      for kt in range(4)]
                        xpairs = [(w_gi_s[:nr, kt, sl], xt[0:nr, kt, 0:BS])
                                  for kt, nr in enumerate(xrows)]
                        if not x_late:
                            pairs += xpairs
                        for (tile_sb, ktb) in extra:
                            for k in range(4):
                                pairs.append((w_gi_s[:, ktb + k, sl],
                                              xa(tile_sb, k)))
                        if x_late:
                            pairs += xpairs
                        emit_group(ps[:, :], pairs)
                        tiles.append(ps)
                    return tiles

                r_ps = gate_wave(0)
                rF = [st.tile([128, BS], F32, tag=f"rF{ht}", bufs=1,
                              name="rF") for ht in range(4)]
                for ht in range(4):
                    nc.scalar.activation(rF[ht][:], r_ps[ht][:, :],
                                         AF.Sigmoid)
                z_ps = gate_wave(1)
                zF = [st.tile([128, BS], F32, tag=f"zF{ht}", bufs=1,
                              name="zF") for ht in range(4)]
                for ht in range(4):
                    nc.scalar.activation(zF[ht][:], z_ps[ht][:, :],
                                         AF.Sigmoid)
                n_ps = gate_wave(2)
                if fill_fn is not None:
                    fill_fn()
                # per-tile chains: t1 = r*bn + n_ps; n = tanh(t1);
                # h' = n + z*(h-n); hT = bf16(h')
                for ht in range(4):
                    t1 = st.tile([128, BS], F32, tag=f"t1{ht}", bufs=1,
                                 name="t1")
                    nc.vector.tensor_mul(t1[:], rF[ht][:], bnF[ht][:])
                    nc.vector.tensor_add(t1[:], t1[:], n_ps[ht][:, :])
                    nF = st.tile([128, BS], F32, tag=f"nF{ht}", bufs=1,
                                 name="nF")
                    nc.scalar.activation(nF[:], t1[:], AF.Tanh)
                    dd = st.tile([128, BS], F32, tag=f"dd{ht}", bufs=1,
                                 name="dd")
                    eng = nc.gpsimd if ht % 2 else nc.vector
                    eng.tensor_sub(dd[:], hFs[ht][:], nF[:])
                    eng.tensor_mul(dd[:], dd[:], zF[ht][:])
                    eng.tensor_add(hFs[ht][:], dd[:], nF[:])
                    nc.vector.tensor_copy(hTs[ht][:], hFs[ht][:])

            # ---------- image projections ----------
            for mt in range(2):
                psv = psp.tile([128, H], F32, tag="gate", name="psv")
                emit_group(psv[:], [(s_imgt[:, kt, mt * 128:(mt + 1) * 128],
                                     s_iv[:, kt, :]) for kt in range(2)])
                nc.scalar.copy(ivv[:, mt, :], psv[:])
            for mt in range(2):
                psik = psp.tile([128, 128], F32, tag="gate", name="psik")
                emit_group(psik[:K, :],
                           [(s_ik[:, kt, :],
                             s_imgt[:, kt, mt * 128:(mt + 1) * 128])
                            for kt in range(2)])
                nc.vector.tensor_copy(ikt2[:K, mt, :], psik[:K, :])

            # ---------- encoder ----------
            with tc.tile_pool(name="qp", bufs=1) as qp:
                s_egi = s_egi_w
                s_egh = s_egh_w
                xt_q = fetch_x(s_qidx, 0)
                # decoder weights: prefetch now, overlapping encoder compute
                s_gi = load(pers, w_gi, BF16)
                s_gh = load(pers, w_gh, BF16)
                s_out = load(pers, w_out, BF16)
                def save_qkqv(ts):
                    # qk/qv projections of step ts (reads current hTs —
                    # must be emitted BEFORE the next h update)
                    for bt in range(2):
                        pb, off = PBS[bt], BOFF[bt]
                        sl = slice(off, off + pb)
                        psk = psp.tile([128, K], F32, tag="gate", name="psk")
                        emit_group(psk[:pb, :],
                                   [(hTs[kt][:, sl], s_qk[:, kt, :])
                                    for kt in range(4)])
                        nc.scalar.copy(qkbs[bt][:pb, ts, :], psk[:pb, :])
                        psv = psp.tile([128, H], F32, tag="gate", name="psv")
                        emit_group(psv[:pb, :],
                                   [(hTs[kt][:, sl], s_qv[:, kt, :])
                                    for kt in range(4)])
                        if bt == 0:
                            nc.scalar.copy(qv_b0[:pb, ts, :], psv[:pb, :])
                        else:
                            g4 = ts % 4
                            nc.scalar.copy(
                                qv_p1[32 * g4:32 * (g4 + 1), ts // 4, :],
                                psv[:pb, :])

                for t in range(L):
                    bnF = gru_bn(s_egh, s_ebhh)

                    def enc_fill(t=t):
                        # PE filler during step t's gate math: next-step
                        # token fetch + the PREVIOUS step's qk/qv (reads
                        # the not-yet-updated hTs = h(t-1))
                        nonlocal xt_q
                        if t + 1 < L:
                            xt_q = fetch_x(s_qidx, t + 1)
                        if t >= 1:
                            save_qkqv(t - 1)

                    gru_fm(s_egi, s_egh, s_ebhh, xt_q, [128, 128, 65], [],
                           bnF, fill_fn=enc_fill)
                save_qkqv(L - 1)

            for i in range(4):
                nc.vector.memset(hTs[i][:], 0.0)
                nc.vector.memset(hFs[i][:], 0.0)

            # ---------- decoder ----------
            with tc.tile_pool(name="lg", bufs=1) as lg:
                o19T = lg.tile([128, 3, BS], FP8)
                nc.vector.memset(o19T[32:64, 2, :], 0.0)
                nc.vector.memset(o19T[64:65, 2, :], 1.0)
                o19_0 = lg.tile([128, D], F32)
                o19_1 = lg.tile([128, D], F32)
                o19_sb = [o19_0, o19_1]

                xt_a = fetch_x(s_aidx, 0)
                at = attn_phase(True)   # for step 0
                for t in range(MAX_LEN):
                    dg2, dg1b = at["dg"]
                    iwT = at["iwT"]

                    bnF = gru_bn(s_gh, s_bhh, act_copies=(t == L))

                    # --- icT (feature-major) ---
                    icT = [wk.tile([128, BS], BF16, tag=f"icT{k}", bufs=2,
                                   name="icT") for k in range(4)]
                    for ht in range(4):
                        ps = psp.tile([128, BS], F32, tag="gate", name="psic")
                        emit_group(ps[:, :],
                                   [(ivv[:, mt, 128 * ht:128 * (ht + 1)],
                                     iwT[:, mt, :]) for mt in range(2)])
                        nc.scalar.activation(icT[ht][:], ps[:, :],
                                             AF.Identity,
                                             bias=s_ivb[:, ht:ht + 1],
                                             scale=1.0)

                    # --- qcT (feature-major, symmetric diag trick) ---
                    qcT = [wk.tile([128, BS], BF16, tag=f"qcT{k}", bufs=2,
                                   name="qcT") for k in range(4)]
                    for ht in range(4):
                        hsl = slice(128 * ht, 128 * (ht + 1))
                        ps = psp.tile([128, BS], F32, tag="gate", name="psqc")
                        first = True
                        for l in range(L):
                            nc.tensor.matmul(ps[:, 0:128],
                                             qv_b0[:, l, hsl],
                                             dg2[:, :, l:l + 1],
                                             start=first, stop=False)
                            first = False
                        for c in range(5):
                            nc.tensor.matmul(ps[:, 128:BS],
                                             qv_p1[:, c, hsl],
                                             dg1b[:, :, c:c + 1],
                                             start=False, stop=(c == 4))
                        if ht % 2 or t == L:
                            nc.scalar.activation(qcT[ht][:], ps[:, :],
                                                 AF.Identity,
                                                 bias=s_qvb[:, ht:ht + 1],
                                                 scale=1.0)
                        else:
                            nc.vector.tensor_scalar_add(qcT[ht][:], ps[:, :],
                                                        s_qvb[:, ht:ht + 1])

                    # --- GRU ---
                    xsrc = xt_a if t < L else dec20
                    gru_fm(s_gi, s_gh, s_bhh, xsrc, [128, 128, 65],
                           [(icT, 7), (qcT, 3)], bnF, x_late=(t == L))
                    if t < L - 1:
                        xt_a = fetch_x(s_aidx, t + 1)

                    # --- attention phase for next step (overlaps out-proj) ---
                    if t + 1 < MAX_LEN:
                        at = attn_phase(True)

                    # --- output projection ---
                    if t != MAX_LEN - 2:
                        osbT = st.tile([128, 3, BS], F32, tag="osbT",
                                       name="osbT")
                        nc.vector.memset(osbT[32:64, 2, :], 0.0)
                        nc.vector.memset(osbT[64:, 2, :], 0.0)
                        for dt_ in range(3):
                            nd = 128 if dt_ < 2 else D - 256
                            sl = slice(128 * dt_, 128 * dt_ + nd)
                            ps = psp.tile([128, BS], F32, tag="gate",
                                          name="pso")
                            pairs = [(s_out[:, 4 + k, sl], qcT[k][:, 0:BS])
                                     for k in range(4)]
                            pairs += [(s_out[:, 8 + k, sl], icT[k][:, 0:BS])
                                      for k in range(4)]
                            pairs.append((s_outb[0:1, sl], ones_bf[0:1, :]))
                            pairs += [(s_out[:, k, sl], hTs[k][:, :])
                                      for k in range(4)]
                            n = len(pairs)
                            for i, (lh, rh) in enumerate(pairs):
                                nc.tensor.matmul(ps[:nd, :], lh, rh,
                                                 start=(i == 0),
                                                 stop=(i == n - 1))
                            nc.vector.tensor_copy(osbT[:nd, dt_, :],
                                                  ps[:nd, :])
                        nc.sync.dma_start(out_o[t], osbT[:])
                    else:
                        # t == 19: batch-major out for argmax rescoring
                        for bt in range(2):
                            pb, off = PBS[bt], BOFF[bt]
                            sl = slice(off, off + pb)
                            pso = psp.tile([128, D], F32, tag="gate",
                                           name="pso19")
                            pairs = [(hTs[k][:, sl], s_out[:, k, :])
                                     for k in range(4)]
                            pairs += [(qcT[k][:, sl], s_out[:, 4 + k, :])
                                      for k in range(4)]
                            pairs += [(icT[k][:, sl], s_out[:, 8 + k, :])
                                      for k in range(4)]
                            pairs.append((ones_bf[0:1, :pb], s_outb[:]))
                            emit_group(pso[:pb, :], pairs)
                            osb = o19_sb[bt]
                            nc.scalar.copy(osb[:pb], pso[:pb])
                            nc.sync.dma_start(out_o19[:pb, bt, :], osb[:pb])
                            ob = st.tile([128, D], BF16, tag="ob",
                                         name="ob")
                            nc.scalar.copy(ob[:pb], osb[:pb])
                            for kt in range(3):
                                w = 128 if kt < 2 else D - 256
                                p8t = psp.tile([128, 128], BF16, tag="trp",
                                               bufs=2, name="p8t")
                                nc.tensor.transpose(p8t[:w, :pb],
                                                    ob[:pb,
                                                       kt * 128:kt * 128 + w],
                                                    ident_bf[:pb, :pb])
                                nc.vector.tensor_copy(
                                    o19T[:w, kt, off:off + pb], p8t[:w, :pb])

                    # --- argmax + re-embed for step 20 ---
                    if t == MAX_LEN - 2:
                        mx18 = [lg.tile([128, 144], F32, name="mx18a"),
                                lg.tile([128, 144], F32, name="mx18b")]
                        ix18 = [lg.tile([128, 144], F32, name="ix18a"),
                                lg.tile([128, 144], F32, name="ix18b")]
                        for nci in range(18):
                            ncw = 512 if nci < 17 else V - 17 * 512
                            rhs = wk.tile([128, 3, 512], FP8, tag="lrhs",
                                          bufs=4, name="rhs")
                            for kt in range(3):
                                nr = 128 if kt < 2 else 65
                                nc.sync.dma_start(
                                    rhs[:nr, kt, :ncw],
                                    embt_bf[:nr, kt,
                                            nci * 512:nci * 512 + ncw])
                            for bt in range(2):
                                pb, off = PBS[bt], BOFF[bt]
                                psl = psp.tile([128, H], F32, tag="gate",
                                               name="psl")
                                if ncw < 512:
                                    nc.vector.memset(psl[:pb, ncw:], NEG)
                                nc.tensor.matmul(
                                    psl[:pb, :ncw],
                                    o19T[:, 0:2, off:off + pb],
                                    rhs[:, 0:2, :ncw],
                                    start=True, stop=False,
                                    perf_mode=PM.DoubleRow)
                                nc.tensor.matmul(
                                    psl[:pb, :ncw],
                                    o19T[:65, 2, off:off + pb],
                                    rhs[:65, 2, :ncw],
                                    start=False, stop=True)
                                nc.vector.max(
                                    mx18[bt][:pb, 8 * nci:8 * nci + 8],
                                    psl[:pb, :])
                                ixc = st.tile([128, 8], U32, tag="ixc",
                                              bufs=3, name="ixc")
                                nc.vector.max_index(
                                    ixc[:pb],
                                    mx18[bt][:pb, 8 * nci:8 * nci + 8],
                                    psl[:pb, :])
                                nc.vector.tensor_copy(
                                    ix18[bt][:pb, 8 * nci:8 * nci + 8],
                                    ixc[:pb])
                        g8s = {}
                        for bt in range(2):
                            pb, off = PBS[bt], BOFF[bt]
                            ixg = st.tile([128, 144], F32, tag=f"ixg{bt}",
                                          name="ixg")
                            nc.vector.tensor_add(ixg[:pb], ix18[bt][:pb],
                                                 off18[:pb])
                            c8 = st.tile([128, 8], F32, tag=f"c8{bt}",
                                         name="c8")
                            nc.vector.max(c8[:pb], mx18[bt][:pb])
                            p8 = st.tile([128, 8], U32, tag=f"p8{bt}",
                                         name="p8")
                            nc.vector.max_index(p8[:pb], c8[:pb],
                                                mx18[bt][:pb])
                            p8f = st.tile([128, 8], F32, tag=f"p8f{bt}",
                                          name="p8f")
                            nc.vector.tensor_copy(p8f[:pb], p8[:pb])
                            ix8f = st.tile([128, 8], F32, tag=f"ix8f{bt}",
                                           name="ix8f")
                            g8s[bt] = []
                            for j in range(8):
                                oh = wk.tile([128, 144], F32, tag="oh144",
                                             bufs=2, name="oh")
                                nc.vector.tensor_scalar(
                                    out=oh[:pb], in0=iota144[:pb],
                                    scalar1=p8f[:pb, j:j + 1], scalar2=None,
                                    op0=ALU.is_equal)
                                nc.vector.tensor_mul(oh[:pb], oh[:pb],
                                                     ixg[:pb])
                                vj = st.tile([128, 1], F32, tag="vj",
                                             name="vj")
                                nc.vector.tensor_reduce(vj[:pb], oh[:pb],
                                                        axis=AX.X, op=ALU.add)
                                nc.vector.tensor_copy(ix8f[:pb, j:j + 1],
                                                      vj[:pb])
                                vju = st.tile([128, 1], U32, tag="vju",
                                              name="vju")
                                nc.vector.tensor_copy(vju[:pb], vj[:pb])
                                g8 = wk.tile([128, D + 1], F32, tag="gath8",
                                             bufs=8, name="g8")
                                nc.gpsimd.indirect_dma_start(
                                    out=g8[:pb], out_offset=None,
                                    in_=emb_aug[:],
                                    in_offset=bass.IndirectOffsetOnAxis(
                                        ap=vju[:pb, 0:1], axis=0))
                                g8s[bt].append(g8)
                            g8s[bt + 2] = ix8f
                        for bt in range(2):
                            pb, off = PBS[bt], BOFF[bt]
                            ix8f = g8s[bt + 2]
                            scores = st.tile([128, 8], F32, tag=f"sc8{bt}",
                                             name="scores")
                            for j in range(8):
                                g8 = g8s[bt][j]
                                pr = wk.tile([128, D], F32, tag="pr8",
                                             name="pr")
                                nc.vector.tensor_mul(pr[:pb], o19_sb[bt][:pb],
                                                     g8[:pb, :D])
                                sj = st.tile([128, 1], F32, tag="sj",
                                             name="sj")
                                nc.vector.tensor_reduce(sj[:pb], pr[:pb],
                                                        axis=AX.X, op=ALU.add)
                                nc.vector.tensor_add(scores[:pb, j:j + 1],
                                                     sj[:pb],
                                                     g8[:pb, D:D + 1])
                            m1 = st.tile([128, 8], F32, tag="m1", name="m1")
                            nc.vector.max(m1[:pb], scores[:pb])
                            j1 = st.tile([128, 8], U32, tag="j1", name="j1")
                            nc.vector.max_index(j1[:pb], m1[:pb],
                                                scores[:pb])
                            j1f = st.tile([128, 1], F32, tag="j1f",
                                          name="j1f")
                            nc.vector.tensor_copy(j1f[:pb], j1[:pb, 0:1])
                            oh8 = st.tile([128, 8], F32, tag="oh8",
                                          name="oh8")
                            nc.vector.tensor_scalar(out=oh8[:pb],
                                                    in0=iota8[:pb],
                                                    scalar1=j1f[:pb],
                                                    scalar2=None,
                                                    op0=ALU.is_equal)
                            nc.vector.tensor_mul(ix8f[:pb], oh8[:pb],
                                                 ix8f[:pb])
                            vsum = st.tile([128, 1], F32, tag="vsum",
                                           name="vsum")
                            nc.vector.tensor_reduce(vsum[:pb], ix8f[:pb],
                                                    axis=AX.X, op=ALU.add)
                            vidx = st.tile([128, 1], U32, tag="vidx",
                                           name="vidx")
                            nc.vector.tensor_copy(vidx[:pb], vsum[:pb])
                            gm = wk.tile([128, D], BF16, tag="gath", bufs=6,
                                         name="gm")
                            nc.gpsimd.indirect_dma_start(
                                out=gm[:pb], out_offset=None, in_=emb_bf[:],
                                in_offset=bass.IndirectOffsetOnAxis(
                                    ap=vidx[:pb, 0:1], axis=0))
                            for kt in range(3):
                                w = 128 if kt < 2 else D - 256
                                tr(dec20[:w, kt, off:off + pb],
                                   gm[:pb, kt * 128:kt * 128 + w], pb, w)

    nc.compile()
    return nc


_NC_CACHE = None


def _get_nc():
    global _NC_CACHE
    if _NC_CACHE is None:
        _NC_CACHE = build_nc()
    return _NC_CACHE


def _pad_tiles(a, ntiles):
    rows, cols = a.shape
    out = np.zeros((128 * ntiles, cols), a.dtype)
    out[:rows] = a
    return np.ascontiguousarray(
        out.reshape(ntiles, 128, cols).transpose(1, 0, 2))


def _prep_shared(inputs):
    bf = np.float16
    f32 = np.float32
    eW = np.asarray(inputs["embed_W"], f32)
    d = {}
    wih = np.asarray(inputs["dec_W_ih"], f32)
    bih = np.asarray(inputs["dec_b_ih"], f32)
    bhh = np.asarray(inputs["dec_b_hh"], f32)
    gi = np.zeros((128 * 11, 3 * H), f32)
    gi[0:D] = wih[:, 0:D].T
    gi[320] = bih + np.concatenate([bhh[:2 * H], np.zeros(H, f32)])
    gi[384:384 + H] = wih[:, D:D + H].T
    gi[896:896 + H] = wih[:, D + H:].T
    d["w_gi"] = _pad_tiles(gi.astype(bf), 11)
    d["w_gh"] = _pad_tiles(np.asarray(inputs["dec_W_hh"], f32).T.astype(bf), 4)
    d["bhh_n"] = np.ascontiguousarray(bhh[2 * H:].astype(bf)[None, :])
    ewih = np.asarray(inputs["enc_W_ih"], f32)
    ebih = np.asarray(inputs["enc_b_ih"], f32)
    ebhh = np.asarray(inputs["enc_b_hh"], f32)
    egi = np.zeros((128 * 3, 3 * H), f32)
    egi[0:D] = ewih[:, :D].T
    egi[320] = ebih + np.concatenate([ebhh[:2 * H], np.zeros(H, f32)])
    d["w_egi"] = _pad_tiles(egi.astype(bf), 3)
    d["w_egh"] = _pad_tiles(np.asarray(inputs["enc_W_hh"], f32).T.astype(bf), 4)
    d["ebhh_n"] = np.ascontiguousarray(ebhh[2 * H:].astype(bf)[None, :])
    d["w_out"] = _pad_tiles(np.asarray(inputs["out_W"], f32).T.astype(bf), 12)
    d["outb"] = np.ascontiguousarray(
        np.asarray(inputs["out_b"], f32).astype(bf)[None, :])
    d["w_qk"] = _pad_tiles(np.asarray(inputs["qk_W"], f32).T.astype(bf), 4)
    d["w_qv"] = _pad_tiles(np.asarray(inputs["qv_W"], f32).T.astype(bf), 4)
    d["qvb_c"] = np.ascontiguousarray(
        np.asarray(inputs["qv_b"], f32).reshape(4, 128).T)
    d["w_ak"] = _pad_tiles(np.asarray(inputs["ak_W"], f32).T.astype(bf), 4)
    d["akb"] = np.ascontiguousarray(
        np.asarray(inputs["ak_b"], f32).astype(bf)[None, :])
    d["w_ik"] = _pad_tiles(np.asarray(inputs["ik_W"], f32).T.astype(bf), 2)
    d["w_iv"] = _pad_tiles(np.asarray(inputs["iv_W"], f32).T.astype(bf), 2)
    d["ivb_c"] = np.ascontiguousarray(
        np.asarray(inputs["iv_b"], f32).reshape(4, 128).T)
    d["emb_bf"] = eW.astype(bf)
    wd_b = np.asarray(inputs["wd_b"], f32)
    d["emb_aug"] = np.ascontiguousarray(
        np.concatenate([eW, wd_b[:, None]], 1))
    aug = np.zeros((128 * 3, VP), f32)
    aug[:D, :V] = eW.T
    aug[320, :V] = wd_b
    import ml_dtypes
    d["embt_bf"] = _pad_tiles(aug.astype(ml_dtypes.float8_e4m3fn), 3)
    return d


def _idx_cols(seq_rows):
    out = np.zeros((128, 2 * L), np.uint32)
    for t in range(L):
        out[:, 2 * t] = seq_rows[0:128, t]
        out[:32, 2 * t + 1] = seq_rows[128:160, t]
    return out


def _build_maps(inputs, shared):
    f32 = np.float32
    bf = np.float16
    ques = np.asarray(inputs["ques_seqs"]).astype(np.uint32)
    ans = np.asarray(inputs["ans_seqs"]).astype(np.uint32)
    qlens = np.asarray(inputs["ques_lens"]).astype(np.int64)
    img = np.asarray(inputs["img_seqs"], f32)
    maps = []
    for s in range(NCORES):
        m = dict(shared)
        r0 = s * BS
        m["q_idx"] = _idx_cols(ques[r0:r0 + BS, :L])
        m["a_idx"] = _idx_cols(ans[r0:r0 + BS, :L])
        qm = np.full((128, 2, L), NEG, f32)
        lens = qlens[r0:r0 + BS]
        for bt, (pb, off) in enumerate(zip(PBS, BOFF)):
            for b in range(pb):
                qm[b, bt, :lens[off + b]] = 0.0
        m["qe_mask"] = qm.astype(bf)
        im = np.full((128, 2, 256), NEG, f32)
        for bt, (pb, off) in enumerate(zip(PBS, BOFF)):
            for b in range(pb):
                gimg = (off + b) // ROUNDS
                im[b, bt, gimg * 16:(gimg + 1) * 16] = 0.0
        m["ie_mask"] = im.astype(bf)
        imgs = img[s * 16:(s + 1) * 16].reshape(256, 256)
        it = np.zeros((128 * 2, 256), f32)
        it[:256] = imgs.T
        m["img_t"] = np.ascontiguousarray(
            it.reshape(2, 128, 256).transpose(1, 0, 2)).astype(bf)
        maps.append(m)
    return maps


def kernel(**inputs):
    nc = _get_nc()
    shared = _prep_shared(inputs)
    in_maps = _build_maps(inputs, shared)
    from concourse.bass_utils import run_bass_kernel_spmd
    res = run_bass_kernel_spmd(nc, in_maps, core_ids=list(range(NCORES)))
    outs = []
    for s in range(NCORES):
        o = np.asarray(res.results[s]["out_o"])      # [21, 128, 3, 160]
        o19 = np.asarray(res.results[s]["out_o19"])  # [128, 2, 300]
        full = o.transpose(3, 0, 2, 1).reshape(BS, MAX_LEN, 384)[:, :, :D]
        full = np.ascontiguousarray(full)
        full[:128, MAX_LEN - 2] = o19[:, 0, :]
        full[128:, MAX_LEN - 2] = o19[:32, 1, :]
        outs.append(full)
    return np.concatenate(outs, 0).astype(np.float32)


# revision 6
# speedup vs baseline: 1.0575x; 1.0035x over previous
"""Trainium2 Bass kernel for nn_BaselineAttnDecoder — feature-major,
software-pipelined.

Per core: 160 decode rows (16 images x 10 rounds), weights replicated.

- GRU gates / out-proj / icT / qcT all FEATURE-major: out [feat<=128, 160]
  PSUM groups, moving dim = true batch 160 (no 128+32 padding waste).
- h lives only as per-tile hT (bf16) + hF (f32): no h transposes.
- qcT via symmetric diag-trick (lhsT = batch-major q_value, rhs = the
  qw diagonal) — attention-weighted sum lands directly feature-major.
- Attention score chain (a, qe, iw softmax, diag build) for step t+1 is
  emitted in step t's tail so the DVE chain overlaps PE matmul work.
- Gate elementwise math runs per h-tile, pipelined across Act/DVE/Pool.
- Step-19 vocab argmax: blockwise top-8 from PSUM chunks (bf16 copies on
  Act, Max/MaxIndex on DVE), exact-f32 rescore of 8 candidates.
"""
import numpy as np

import concourse.bass as bass
import concourse.bacc as bacc
import concourse.mybir as mybir
import concourse.tile as tile
from concourse.masks import make_identity

F32 = mybir.dt.float32
BF16 = mybir.dt.float16  # 16-bit compute dtype (f16: 10-bit mantissa)
U32 = mybir.dt.uint32
FP8 = mybir.dt.float8e4
PM = mybir.MatmulPerfMode
AF = mybir.ActivationFunctionType
ALU = mybir.AluOpType
AX = mybir.AxisListType

D, H, V, K = 300, 512, 8835, 50
L, MAX_LEN, ROUNDS = 20, 21, 10
BS = 160
NCORES = 8
PBS = [128, 32]
BOFF = [0, 128]
VP = 18 * 512
NEG = -60000.0


def bcast_mid(ap, reps):
    return bass.AP(tensor=ap.tensor, offset=ap.offset,
                   ap=[ap.ap[0], [0, reps], ap.ap[1]])


def bcast_in(ap, reps):
    return bass.AP(tensor=ap.tensor, offset=ap.offset,
                   ap=[ap.ap[0], ap.ap[1], [0, reps]])


def build_nc():
    nc = bacc.Bacc()

    def din(name, shape, dt):
        return nc.dram_tensor(name, shape, dt, kind="ExternalInput")

    w_gi = din("w_gi", [128, 11, 3 * H], BF16)
    w_gh = din("w_gh", [128, 4, 3 * H], BF16)
    bhh_n = din("bhh_n", [1, H], BF16)
    w_egi = din("w_egi", [128, 3, 3 * H], BF16)
    w_egh = din("w_egh", [128, 4, 3 * H], BF16)
    ebhh_n = din("ebhh_n", [1, H], BF16)
    w_out = din("w_out", [128, 12, D], BF16)
    outb = din("outb", [1, D], BF16)
    w_qk = din("w_qk", [128, 4, K], BF16)
    w_qv = din("w_qv", [128, 4, H], BF16)
    w_ak = din("w_ak", [128, 4, K], BF16)
    akb = din("akb", [1, K], BF16)
    w_ik = din("w_ik", [128, 2, K], BF16)
    w_iv = din("w_iv", [128, 2, H], BF16)
    ivb_c = din("ivb_c", [128, 4], F32)
    qvb_c = din("qvb_c", [128, 4], F32)
    img_t = din("img_t", [128, 2, 2 * 128], BF16)
    emb_bf = din("emb_bf", [V, D], BF16)
    emb_aug = din("emb_aug", [V, D + 1], F32)
    embt_bf = din("embt_bf", [128, 3, VP], FP8)
    q_idx = din("q_idx", [128, 2 * L], U32)
    a_idx = din("a_idx", [128, 2 * L], U32)
    qe_mask = din("qe_mask", [128, 2, L], BF16)
    ie_mask = din("ie_mask", [128, 2, 2 * 128], BF16)

    out_o = nc.dram_tensor("out_o", [MAX_LEN, 128, 3, BS], F32,
                           kind="ExternalOutput")
    out_o19 = nc.dram_tensor("out_o19", [128, 2, D], F32,
                             kind="ExternalOutput")

    with tile.TileContext(nc) as tc:
        with (
            tc.tile_pool(name="cw", bufs=1) as cw,
            tc.tile_pool(name="pers", bufs=1) as pers,
            tc.tile_pool(name="wk", bufs=2) as wk,
            tc.tile_pool(name="st", bufs=2) as st,
            tc.tile_pool(name="ps", bufs=6, space="PSUM") as psp,
        ):
            def load(pool, t, dt):
                s = pool.tile(list(t.shape), dt, name=t.name + "_sb")
                nc.sync.dma_start(s[:], t[:])
                return s

            # encoder-critical loads first (SP queue is in-order)
            s_qidx = load(cw, q_idx, U32)
            s_egi_w = load(cw, w_egi, BF16)
            s_egh_w = load(cw, w_egh, BF16)
            s_ebhh = load(cw, ebhh_n, BF16)
            s_qk = load(cw, w_qk, BF16)
            s_qv = load(cw, w_qv, BF16)
            s_ak = load(cw, w_ak, BF16)
            s_ik = load(cw, w_ik, BF16)
            s_iv = load(cw, w_iv, BF16)
            s_imgt = load(cw, img_t, BF16)
            s_bhh = load(cw, bhh_n, BF16)
            s_outb = load(cw, outb, BF16)
            s_akb = load(cw, akb, BF16)
            s_ivb = load(cw, ivb_c, F32)
            s_qvb = load(cw, qvb_c, F32)
            s_aidx = load(cw, a_idx, U32)
            s_qem = load(cw, qe_mask, BF16)
            s_iem = load(cw, ie_mask, BF16)

            ident_bf = cw.tile([128, 128], BF16)
            make_identity(nc, ident_bf[:])
            ones_bf = cw.tile([1, BS], BF16)
            nc.vector.memset(ones_bf[:], 1.0)
            sid4 = cw.tile([128, 32], BF16)
            for g4 in range(4):
                nc.vector.tensor_copy(sid4[32 * g4:32 * (g4 + 1), :],
                                      ident_bf[0:32, 0:32])
            iota8 = cw.tile([128, 8], F32)
            nc.gpsimd.iota(iota8[:], pattern=[[1, 8]], base=0,
                           channel_multiplier=0,
                           allow_small_or_imprecise_dtypes=True)
            iota144 = cw.tile([128, 144], F32)
            nc.gpsimd.iota(iota144[:], pattern=[[1, 144]], base=0,
                           channel_multiplier=0,
                           allow_small_or_imprecise_dtypes=True)
            off18 = cw.tile([128, 144], F32)
            nc.gpsimd.iota(off18[:], pattern=[[512, 18], [0, 8]], base=0,
                           channel_multiplier=0,
                           allow_small_or_imprecise_dtypes=True)
            # identity replicated along an inner L/5 axis (for diag builds
            # that keep innermost stride-1 so DVE 2x mode applies)
            i_rep = cw.tile([128, 128, L], BF16)
            for l in range(L):
                nc.vector.tensor_copy(i_rep[:, :, l:l + 1],
                                      bass.AP(tensor=ident_bf.tensor,
                                              offset=ident_bf[:, :].offset,
                                              ap=[ident_bf[:, :].ap[0],
                                                  [1, 128], [0, 1]]))
            sid_rep = cw.tile([128, 32, 5], BF16)
            for c in range(5):
                nc.vector.tensor_copy(sid_rep[:, :, c:c + 1],
                                      bass.AP(tensor=sid4.tensor,
                                              offset=sid4[:, :].offset,
                                              ap=[sid4[:, :].ap[0],
                                                  [1, 32], [0, 1]]))

            # persistent state — h per tile, feature-major
            hTs = [pers.tile([128, BS], BF16, name=f"hT{i}") for i in range(4)]
            hFs = [pers.tile([128, BS], F32, name=f"hF{i}") for i in range(4)]
            qk_b0 = pers.tile([128, L, K], BF16)
            qk_b1 = pers.tile([128, L, K], BF16)
            qkbs = [qk_b0, qk_b1]
            qv_b0 = pers.tile([128, L, H], BF16)
            qv_p1 = pers.tile([128, 5, H], BF16)
            ivv = pers.tile([128, 2, H], BF16)
            ikt2 = pers.tile([128, 2, 128], BF16)
            dec20 = pers.tile([128, 3, BS], BF16)

            for i in range(4):
                nc.vector.memset(hTs[i][:], 0.0)
                nc.vector.memset(hFs[i][:], 0.0)
            nc.vector.memset(dec20[32:64, 2, :], 0.0)
            nc.vector.memset(dec20[64:65, 2, :], 1.0)

            def tr(dst_sb_ap, src_sb_ap, pb, w, eng=None):
                p = psp.tile([128, 128], BF16, tag="trp", bufs=2, name="pt")
                nc.tensor.transpose(p[:w, :pb], src_sb_ap, ident_bf[:pb, :pb])
                (eng or nc.vector).tensor_copy(dst_sb_ap, p[:w, :pb])

            def fetch_x(idx_sb, t):
                xt = wk.tile([128, 3, BS], BF16, tag="xt", bufs=3, name="xt")
                nc.vector.memset(xt[32:64, 2, :], 0.0)
                nc.vector.memset(xt[64:65, 2, :], 1.0)
                for c, (pb, off) in enumerate(zip(PBS, BOFF)):
                    g = wk.tile([128, D], BF16, tag="gath", bufs=6, name="g")
                    nc.gpsimd.indirect_dma_start(
                        out=g[:pb], out_offset=None, in_=emb_bf[:],
                        in_offset=bass.IndirectOffsetOnAxis(
                            ap=idx_sb[:pb, 2 * t + c:2 * t + c + 1], axis=0))
                    for kt in range(3):
                        w = 128 if kt < 2 else D - 256
                        if kt == 1:
                            p = psp.tile([128, 128], BF16, tag="trp", bufs=2,
                                         name="pt")
                            nc.tensor.transpose(p[:w, :pb],
                                                g[:pb, 128:128 + w],
                                                ident_bf[:pb, :pb])
                            nc.scalar.copy(xt[:w, kt, off:off + pb],
                                           p[:w, :pb])
                        else:
                            tr(xt[:w, kt, off:off + pb],
                               g[:pb, kt * 128:kt * 128 + w], pb, w)
                return xt

            def emit_group(ps_ap, pairs):
                n = len(pairs)
                for i, (lh, rh) in enumerate(pairs):
                    nc.tensor.matmul(ps_ap, lh, rh, start=(i == 0),
                                     stop=(i == n - 1))

            # ---------- attention-score phase for step t (emitted in the
            # tail of step t-1; depends only on hTs) ----------
            def attn_phase(dec):
                """Returns dict with qw diag tiles + iwT for the next step."""
                r = {}
                # a = ak(h) + akb (batch-major), aT
                a_bf = st.tile([128, 2, K], BF16, tag="a_bf", name="a_bf")
                aT = st.tile([128, BS], BF16, tag="aT", name="aT")
                for bt in range(2):
                    pb, off = PBS[bt], BOFF[bt]
                    sl = slice(off, off + pb)
                    psa = psp.tile([128, K], F32, tag="gate", name="psa")
                    pairs = [(hTs[kt][:, sl], s_ak[:, kt, :])
                             for kt in range(4)]
                    pairs.append((ones_bf[0:1, :pb], s_akb[:]))
                    emit_group(psa[:pb, :], pairs)
                    nc.scalar.copy(a_bf[:pb, bt, :], psa[:pb, :])
                    tr(aT[:K, off:off + pb], a_bf[:pb, bt, :], pb, K)
                if not dec:
                    return r
                # question attention softmax -> normalized diag tiles.
                # dg2[b', b, l] = ew[b', l] * rs[b'] * I[b', b] in ONE
                # scalar_tensor_tensor (2x mode: all innermost stride-1).
                dg2 = wk.tile([128, 128, L], BF16, tag="dg2", bufs=2,
                              name="dg2")
                dg1b = wk.tile([128, 32, 5], BF16, tag="dg1b", bufs=2,
                              name="dg1b")
                ews = []
                rss = []
                for bt in range(2):
                    pb = PBS[bt]
                    prod = wk.tile([128, L, K], BF16, tag="prod", bufs=2,
                                   name="prod")
                    nc.vector.tensor_mul(prod[:pb], qkbs[bt][:pb],
                                         bcast_mid(a_bf[:pb, bt, :], L))
                    qe = st.tile([128, L], BF16, tag="qe", name="qe")
                    with nc.allow_low_precision(reason="attn scores bf16"):
                        nc.vector.tensor_reduce(qe[:pb], prod[:pb],
                                                axis=AX.X, op=ALU.add)
                    nc.vector.tensor_add(qe[:pb], qe[:pb], s_qem[:pb, bt, :])
                    nm = st.tile([128, 1], F32, tag="nm", name="nm")
                    nc.vector.tensor_reduce(nm[:pb], qe[:pb], axis=AX.X,
                                            op=ALU.max, negate=True)
                    ew = st.tile([128, L], BF16, tag="ew", name="ew")
                    ssum = st.tile([128, 1], F32, tag="ssum", name="ssum")
                    nc.scalar.activation(ew[:pb], qe[:pb], AF.Exp,
                                         bias=nm[:pb], scale=1.0,
                                         accum_out=ssum[:pb])
                    rs = st.tile([128, 1], F32, tag="rs", name="rs")
                    nc.vector.reciprocal(rs[:pb], ssum[:pb])
                    ews.append(ew)
                    rss.append(rs)
                qwn = st.tile([128, L], BF16, tag="qwn", name="qwn")
                nc.vector.tensor_scalar_mul(qwn[:128, :], ews[0][:128, :],
                                            rss[0][:128, :])
                nc.vector.tensor_mul(dg2[:, :, :],
                                     bcast_mid(qwn[:128, :], 128),
                                     i_rep[:, :, :])
                ew_pk = st.tile([128, 5], BF16, tag="ew_pk", name="ew_pk")
                for g4 in range(4):
                    nc.vector.tensor_scalar_mul(
                        ew_pk[32 * g4:32 * (g4 + 1), :],
                        ews[1][0:32, g4:L:4], rss[1][0:32, :])
                nc.vector.tensor_mul(dg1b[:, :, :],
                                     bcast_mid(ew_pk[:, :], 32),
                                     sid_rep[:, :, :])
                r["dg"] = (dg2, dg1b)
                # image attention softmax -> iwT
                iwT = st.tile([128, 2, BS], BF16, tag="iwT", name="iwT")
                for bt in range(2):
                    pb, off = PBS[bt], BOFF[bt]
                    psi = psp.tile([128, 256], F32, tag="gate", name="psi")
                    nc.tensor.matmul(psi[:pb, :], aT[:K, off:off + pb],
                                     ikt2[:K, :, :], start=True, stop=True)
                    iem = st.tile([128, 256], BF16, tag="iem", name="iem")
                    with nc.allow_low_precision(reason="attn scores bf16"):
                        nc.vector.tensor_add(iem[:pb], psi[:pb],
                                             s_iem[:pb, bt, :])
                    nmi = st.tile([128, 1], F32, tag="nmi", name="nmi")
                    nc.vector.tensor_reduce(nmi[:pb], iem[:pb], axis=AX.X,
                                            op=ALU.max, negate=True)
                    ewi = st.tile([128, 256], BF16, tag="ewi", name="ewi")
                    ssi = st.tile([128, 1], F32, tag="ssi", name="ssi")
                    nc.scalar.activation(ewi[:pb], iem[:pb], AF.Exp,
                                         bias=nmi[:pb], scale=1.0,
                                         accum_out=ssi[:pb])
                    rsi = st.tile([128, 1], F32, tag="rsi", name="rsi")
                    nc.vector.reciprocal(rsi[:pb], ssi[:pb])
                    drs = st.tile([128, 128], BF16, tag="drs", name="drs")
                    nc.vector.tensor_scalar_mul(drs[:pb, :pb],
                                                ident_bf[:pb, :pb],
                                                rsi[:pb])
                    for c in range(2):
                        p = psp.tile([128, 128], F32, tag="trp", bufs=2,
                                     name="ptw")
                        nc.tensor.matmul(p[:128, :pb],
                                         ewi[:pb, c * 128:(c + 1) * 128],
                                         drs[:pb, :pb],
                                         start=True, stop=True)
                        nc.vector.tensor_copy(iwT[:, c, off:off + pb],
                                              p[:128, :pb])
                r["iwT"] = iwT
                return r

            # ---------- feature-major GRU core ----------
            def gru_bn(w_gh_s, bhh_s, act_copies=False):
                """BN wave: gh_n x h + bhh_n. Depends only on hTs — emit as
                early as possible in the step."""
                bn_ps = []
                for ht in range(4):
                    sl = slice(2 * H + 128 * ht, 2 * H + 128 * (ht + 1))
                    ps = psp.tile([128, BS], F32, tag="gate", name="bn")
                    pairs = [(w_gh_s[:, kt, sl], hTs[kt][:, :])
                             for kt in range(4)]
                    pairs.append((bhh_s[0:1, 128 * ht:128 * (ht + 1)],
                                  ones_bf[0:1, :]))
                    emit_group(ps[:, :], pairs)
                    bn_ps.append(ps)
                bnF = [st.tile([128, BS], F32, tag=f"bnF{ht}", bufs=1,
                               name="bnF") for ht in range(4)]
                for ht in range(4):
                    (nc.scalar.copy if (act_copies or ht % 2) else
                     nc.vector.tensor_copy)(bnF[ht][:], bn_ps[ht][:, :])
                return bnF

            def gru_fm(w_gi_s, w_gh_s, bhh_s, xt, xrows, extra, bnF,
                       x_late=False, fill_fn=None):
                """extra: list of (sbuf_tile_or_list, kt_base). Updates
                hTs/hFs in place. Gate math pipelined per h-tile."""
                def xa(tile_sb, k):
                    if isinstance(tile_sb, list):
                        return tile_sb[k][:, 0:BS]
                    return tile_sb[:, k, 0:BS]

                def gate_wave(ci):
                    tiles = []
                    for ht in range(4):
                        sl = slice(ci * H + 128 * ht, ci * H + 128 * (ht + 1))
                        ps = psp.tile([128, BS], F32, tag="gate",
                                      name=f"g{ci}")
                        pairs = []
                        if ci < 2:
                            pairs += [(w_gh_s[:, kt, sl], hTs[kt][:, :])
                                      for kt in range(4)]
                        xpairs = [(w_gi_s[:nr, kt, sl], xt[0:nr, kt, 0:BS])
                                  for kt, nr in enumerate(xrows)]
                        if not x_late:
                            pairs += xpairs
                        for (tile_sb, ktb) in extra:
                            for k in range(4):
                                pairs.append((w_gi_s[:, ktb + k, sl],
                                              xa(tile_sb, k)))
                        if x_late:
                            pairs += xpairs
                        emit_group(ps[:, :], pairs)
                        tiles.append(ps)
                    return tiles

                r_ps = gate_wave(0)
                rF = [st.tile([128, BS], F32, tag=f"rF{ht}", bufs=1,
                              name="rF") for ht in range(4)]
                for ht in range(4):
                    nc.scalar.activation(rF[ht][:], r_ps[ht][:, :],
                                         AF.Sigmoid)
                z_ps = gate_wave(1)
                zF = [st.tile([128, BS], F32, tag=f"zF{ht}", bufs=1,
                              name="zF") for ht in range(4)]
                for ht in range(4):
                    nc.scalar.activation(zF[ht][:], z_ps[ht][:, :],
                                         AF.Sigmoid)
                n_ps = gate_wave(2)
                if fill_fn is not None:
                    fill_fn()
                # per-tile chains: t1 = r*bn + n_ps; n = tanh(t1);
                # h' = n + z*(h-n); hT = bf16(h')
                for ht in range(4):
                    t1 = st.tile([128, BS], F32, tag=f"t1{ht}", bufs=1,
                                 name="t1")
                    nc.vector.tensor_mul(t1[:], rF[ht][:], bnF[ht][:])
                    nc.vector.tensor_add(t1[:], t1[:], n_ps[ht][:, :])
                    nF = st.tile([128, BS], F32, tag=f"nF{ht}", bufs=1,
                                 name="nF")
                    nc.scalar.activation(nF[:], t1[:], AF.Tanh)
                    dd = st.tile([128, BS], F32, tag=f"dd{ht}", bufs=1,
                                 name="dd")
                    eng = nc.gpsimd if ht % 2 else nc.vector
                    eng2 = nc.vector if ht % 2 else nc.gpsimd
                    eng.tensor_sub(dd[:], hFs[ht][:], nF[:])
                    eng.tensor_mul(dd[:], dd[:], zF[ht][:])
                    # bf16 hT produced in parallel with the f32 master add
                    with nc.allow_low_precision(reason="hT bf16 copy"):
                        eng2.tensor_add(hTs[ht][:], dd[:], nF[:])
                    eng.tensor_add(hFs[ht][:], dd[:], nF[:])

            # ---------- image projections ----------
            for mt in range(2):
                psv = psp.tile([128, H], F32, tag="gate", name="psv")
                emit_group(psv[:], [(s_imgt[:, kt, mt * 128:(mt + 1) * 128],
                                     s_iv[:, kt, :]) for kt in range(2)])
                nc.scalar.copy(ivv[:, mt, :], psv[:])
            for mt in range(2):
                psik = psp.tile([128, 128], F32, tag="gate", name="psik")
                emit_group(psik[:K, :],
                           [(s_ik[:, kt, :],
                             s_imgt[:, kt, mt * 128:(mt + 1) * 128])
                            for kt in range(2)])
                nc.vector.tensor_copy(ikt2[:K, mt, :], psik[:K, :])

            # ---------- encoder ----------
            with tc.tile_pool(name="qp", bufs=1) as qp:
                s_egi = s_egi_w
                s_egh = s_egh_w
                xt_q = fetch_x(s_qidx, 0)
                # decoder weights: prefetch now, overlapping encoder compute
                s_gi = load(pers, w_gi, BF16)
                s_gh = load(pers, w_gh, BF16)
                s_out = load(pers, w_out, BF16)
                def save_qkqv(ts):
                    # qk/qv projections of step ts (reads current hTs —
                    # must be emitted BEFORE the next h update)
                    for bt in range(2):
                        pb, off = PBS[bt], BOFF[bt]
                        sl = slice(off, off + pb)
                        psk = psp.tile([128, K], F32, tag="gate", name="psk")
                        emit_group(psk[:pb, :],
                                   [(hTs[kt][:, sl], s_qk[:, kt, :])
                                    for kt in range(4)])
                        nc.scalar.copy(qkbs[bt][:pb, ts, :], psk[:pb, :])
                        psv = psp.tile([128, H], F32, tag="gate", name="psv")
                        emit_group(psv[:pb, :],
                                   [(hTs[kt][:, sl], s_qv[:, kt, :])
                                    for kt in range(4)])
                        if bt == 0:
                            nc.scalar.copy(qv_b0[:pb, ts, :], psv[:pb, :])
                        else:
                            g4 = ts % 4
                            nc.scalar.copy(
                                qv_p1[32 * g4:32 * (g4 + 1), ts // 4, :],
                                psv[:pb, :])

                for t in range(L):
                    bnF = gru_bn(s_egh, s_ebhh)

                    def enc_fill(t=t):
                        # PE filler during step t's gate math: next-step
                        # token fetch + the PREVIOUS step's qk/qv (reads
                        # the not-yet-updated hTs = h(t-1))
                        nonlocal xt_q
                        if t + 1 < L:
                            xt_q = fetch_x(s_qidx, t + 1)
                        if t >= 1:
                            save_qkqv(t - 1)

                    gru_fm(s_egi, s_egh, s_ebhh, xt_q, [128, 128, 65], [],
                           bnF, fill_fn=enc_fill)
                save_qkqv(L - 1)

            for i in range(4):
                nc.vector.memset(hTs[i][:], 0.0)
                nc.vector.memset(hFs[i][:], 0.0)

            # ---------- decoder ----------
            with tc.tile_pool(name="lg", bufs=1) as lg:
                o19T = lg.tile([128, 3, BS], FP8)
                nc.vector.memset(o19T[32:64, 2, :], 0.0)
                nc.vector.memset(o19T[64:65, 2, :], 1.0)
                o19_0 = lg.tile([128, D], F32)
                o19_1 = lg.tile([128, D], F32)
                o19_sb = [o19_0, o19_1]

                xt_a = fetch_x(s_aidx, 0)
                at = attn_phase(True)   # for step 0
                for t in range(MAX_LEN):
                    dg2, dg1b = at["dg"]
                    iwT = at["iwT"]

                    bnF = gru_bn(s_gh, s_bhh, act_copies=(t == L))

                    # --- icT (feature-major) ---
                    icT = [wk.tile([128, BS], BF16, tag=f"icT{k}", bufs=2,
                                   name="icT") for k in range(4)]
                    for ht in range(4):
                        ps = psp.tile([128, BS], F32, tag="gate", name="psic")
                        emit_group(ps[:, :],
                                   [(ivv[:, mt, 128 * ht:128 * (ht + 1)],
                                     iwT[:, mt, :]) for mt in range(2)])
                        nc.scalar.activation(icT[ht][:], ps[:, :],
                                             AF.Identity,
                                             bias=s_ivb[:, ht:ht + 1],
                                             scale=1.0)

                    # --- qcT (feature-major, symmetric diag trick) ---
                    qcT = [wk.tile([128, BS], BF16, tag=f"qcT{k}", bufs=2,
                                   name="qcT") for k in range(4)]
                    for ht in range(4):
                        hsl = slice(128 * ht, 128 * (ht + 1))
                        ps = psp.tile([128, BS], F32, tag="gate", name="psqc")
                        first = True
                        for l in range(L):
                            nc.tensor.matmul(ps[:, 0:128],
                                             qv_b0[:, l, hsl],
                                             dg2[:, :, l:l + 1],
                                             start=first, stop=False)
                            first = False
                        for c in range(5):
                            nc.tensor.matmul(ps[:, 128:BS],
                                             qv_p1[:, c, hsl],
                                             dg1b[:, :, c:c + 1],
                                             start=False, stop=(c == 4))
                        if ht % 2 or t == L:
                            nc.scalar.activation(qcT[ht][:], ps[:, :],
                                                 AF.Identity,
                                                 bias=s_qvb[:, ht:ht + 1],
                                                 scale=1.0)
                        else:
                            nc.vector.tensor_scalar_add(qcT[ht][:], ps[:, :],
                                                        s_qvb[:, ht:ht + 1])

                    # --- GRU ---
                    xsrc = xt_a if t < L else dec20
                    gru_fm(s_gi, s_gh, s_bhh, xsrc, [128, 128, 65],
                           [(icT, 7), (qcT, 3)], bnF, x_late=(t == L))
                    if t < L - 1:
                        xt_a = fetch_x(s_aidx, t + 1)

                    # --- attention phase for next step (overlaps out-proj) ---
                    if t + 1 < MAX_LEN:
                        at = attn_phase(True)

                    # --- output projection ---
                    if t != MAX_LEN - 2:
                        osbT = st.tile([128, 3, BS], F32, tag="osbT",
                                       name="osbT")
                        nc.vector.memset(osbT[32:64, 2, :], 0.0)
                        nc.vector.memset(osbT[64:, 2, :], 0.0)
                        for dt_ in range(3):
                            nd = 128 if dt_ < 2 else D - 256
                            sl = slice(128 * dt_, 128 * dt_ + nd)
                            ps = psp.tile([128, BS], F32, tag="gate",
                                          name="pso")
                            pairs = [(s_out[:, 4 + k, sl], qcT[k][:, 0:BS])
                                     for k in range(4)]
                            pairs += [(s_out[:, 8 + k, sl], icT[k][:, 0:BS])
                                      for k in range(4)]
                            pairs.append((s_outb[0:1, sl], ones_bf[0:1, :]))
                            pairs += [(s_out[:, k, sl], hTs[k][:, :])
                                      for k in range(4)]
                            n = len(pairs)
                            for i, (lh, rh) in enumerate(pairs):
                                nc.tensor.matmul(ps[:nd, :], lh, rh,
                                                 start=(i == 0),
                                                 stop=(i == n - 1))
                            nc.vector.tensor_copy(osbT[:nd, dt_, :],
                                                  ps[:nd, :])
                        nc.sync.dma_start(out_o[t], osbT[:])
                    else:
                        # t == 19: batch-major out for argmax rescoring
                        for bt in range(2):
                            pb, off = PBS[bt], BOFF[bt]
                            sl = slice(off, off + pb)
                            pso = psp.tile([128, D], F32, tag="gate",
                                           name="pso19")
                            pairs = [(hTs[k][:, sl], s_out[:, k, :])
                                     for k in range(4)]
                            pairs += [(qcT[k][:, sl], s_out[:, 4 + k, :])
                                      for k in range(4)]
                            pairs += [(icT[k][:, sl], s_out[:, 8 + k, :])
                                      for k in range(4)]
                            pairs.append((ones_bf[0:1, :pb], s_outb[:]))
                            emit_group(pso[:pb, :], pairs)
                            osb = o19_sb[bt]
                            nc.scalar.copy(osb[:pb], pso[:pb])
                            nc.sync.dma_start(out_o19[:pb, bt, :], osb[:pb])
                            ob = st.tile([128, D], BF16, tag="ob",
                                         name="ob")
                            nc.scalar.copy(ob[:pb], osb[:pb])
                            for kt in range(3):
                                w = 128 if kt < 2 else D - 256
                                p8t = psp.tile([128, 128], BF16, tag="trp",
                                               bufs=2, name="p8t")
                                nc.tensor.transpose(p8t[:w, :pb],
                                                    ob[:pb,
                                                       kt * 128:kt * 128 + w],
                                                    ident_bf[:pb, :pb])
                                nc.vector.tensor_copy(
                                    o19T[:w, kt, off:off + pb], p8t[:w, :pb])

                    # --- argmax + re-embed for step 20 ---
                    if t == MAX_LEN - 2:
                        mx18 = [lg.tile([128, 144], F32, name="mx18a"),
                                lg.tile([128, 144], F32, name="mx18b")]
                        ix18 = [lg.tile([128, 144], F32, name="ix18a"),
                                lg.tile([128, 144], F32, name="ix18b")]
                        for nci in range(18):
                            ncw = 512 if nci < 17 else V - 17 * 512
                            rhs = wk.tile([128, 3, 512], FP8, tag="lrhs",
                                          bufs=4, name="rhs")
                            nc.sync.dma_start(
                                rhs[:, :, :ncw],
                                embt_bf[:, :, nci * 512:nci * 512 + ncw])
                            for bt in range(2):
                                pb, off = PBS[bt], BOFF[bt]
                                psl = psp.tile([128, H], F32, tag="gate",
                                               name="psl")
                                if ncw < 512:
                                    nc.vector.memset(psl[:pb, ncw:], NEG)
                                nc.tensor.matmul(
                                    psl[:pb, :ncw],
                                    o19T[:, 0:2, off:off + pb],
                                    rhs[:, 0:2, :ncw],
                                    start=True, stop=False,
                                    perf_mode=PM.DoubleRow)
                                nc.tensor.matmul(
                                    psl[:pb, :ncw],
                                    o19T[:65, 2, off:off + pb],
                                    rhs[:65, 2, :ncw],
                                    start=False, stop=True)
                                nc.vector.max(
                                    mx18[bt][:pb, 8 * nci:8 * nci + 8],
                                    psl[:pb, :])
                                ixc = st.tile([128, 8], U32, tag="ixc",
                                              bufs=3, name="ixc")
                                nc.vector.max_index(
                                    ixc[:pb],
                                    mx18[bt][:pb, 8 * nci:8 * nci + 8],
                                    psl[:pb, :])
                                nc.vector.tensor_copy(
                                    ix18[bt][:pb, 8 * nci:8 * nci + 8],
                                    ixc[:pb])
                        g8s = {}
                        for bt in range(2):
                            pb, off = PBS[bt], BOFF[bt]
                            ixg = st.tile([128, 144], F32, tag=f"ixg{bt}",
                                          name="ixg")
                            nc.vector.tensor_add(ixg[:pb], ix18[bt][:pb],
                                                 off18[:pb])
                            c8 = st.tile([128, 8], F32, tag=f"c8{bt}",
                                         name="c8")
                            nc.vector.max(c8[:pb], mx18[bt][:pb])
                            p8 = st.tile([128, 8], U32, tag=f"p8{bt}",
                                         name="p8")
                            nc.vector.max_index(p8[:pb], c8[:pb],
                                                mx18[bt][:pb])
                            p8f = st.tile([128, 8], F32, tag=f"p8f{bt}",
                                          name="p8f")
                            nc.vector.tensor_copy(p8f[:pb], p8[:pb])
                            ix8f = st.tile([128, 8], F32, tag=f"ix8f{bt}",
                                           name="ix8f")
                            g8s[bt] = []
                            for j in range(8):
                                oh = wk.tile([128, 144], F32, tag="oh144",
                                             bufs=2, name="oh")
                                nc.vector.tensor_scalar(
                                    out=oh[:pb], in0=iota144[:pb],
                                    scalar1=p8f[:pb, j:j + 1], scalar2=None,
                                    op0=ALU.is_equal)
                                nc.vector.tensor_mul(oh[:pb], oh[:pb],
                                                     ixg[:pb])
                                vj = st.tile([128, 1], F32, tag="vj",
                                             name="vj")
                                nc.vector.tensor_reduce(vj[:pb], oh[:pb],
                                                        axis=AX.X, op=ALU.add)
                                nc.vector.tensor_copy(ix8f[:pb, j:j + 1],
                                                      vj[:pb])
                                vju = st.tile([128, 1], U32, tag="vju",
                                              name="vju")
                                nc.vector.tensor_copy(vju[:pb], vj[:pb])
                                g8 = wk.tile([128, D + 1], F32, tag="gath8",
                                             bufs=8, name="g8")
                                nc.gpsimd.indirect_dma_start(
                                    out=g8[:pb], out_offset=None,
                                    in_=emb_aug[:],
                                    in_offset=bass.IndirectOffsetOnAxis(
                                        ap=vju[:pb, 0:1], axis=0))
                                g8s[bt].append(g8)
                            g8s[bt + 2] = ix8f
                        for bt in range(2):
                            pb, off = PBS[bt], BOFF[bt]
                            ix8f = g8s[bt + 2]
                            scores = st.tile([128, 8], F32, tag=f"sc8{bt}",
                                             name="scores")
                            for j in range(8):
                                g8 = g8s[bt][j]
                                pr = wk.tile([128, D], F32, tag="pr8",
                                             name="pr")
                                nc.vector.tensor_mul(pr[:pb], o19_sb[bt][:pb],
                                                     g8[:pb, :D])
                                sj = st.tile([128, 1], F32, tag="sj",
                                             name="sj")
                                nc.vector.tensor_reduce(sj[:pb], pr[:pb],
                                                        axis=AX.X, op=ALU.add)
                                nc.vector.tensor_add(scores[:pb, j:j + 1],
                                                     sj[:pb],
                                                     g8[:pb, D:D + 1])
                            m1 = st.tile([128, 8], F32, tag="m1", name="m1")
                            nc.vector.max(m1[:pb], scores[:pb])
                            j1 = st.tile([128, 8], U32, tag="j1", name="j1")
                            nc.vector.max_index(j1[:pb], m1[:pb],
                                                scores[:pb])
                            j1f = st.tile([128, 1], F32, tag="j1f",
                                          name="j1f")
                            nc.vector.tensor_copy(j1f[:pb], j1[:pb, 0:1])
                            oh8 = st.tile([128, 8], F32, tag="oh8",
                                          name="oh8")
                            nc.vector.tensor_scalar(out=oh8[:pb],
                                                    in0=iota8[:pb],
                                                    scalar1=j1f[:pb],
                                                    scalar2=None,
                                                    op0=ALU.is_equal)
                            nc.vector.tensor_mul(ix8f[:pb], oh8[:pb],
                                                 ix8f[:pb])
                            vsum = st.tile([128, 1], F32, tag="vsum",
                                           name="vsum")
                            nc.vector.tensor_reduce(vsum[:pb], ix8f[:pb],
                                                    axis=AX.X, op=ALU.add)
                            vidx = st.tile([128, 1], U32, tag="vidx",
                                           name="vidx")
                            nc.vector.tensor_copy(vidx[:pb], vsum[:pb])
                            gm = wk.tile([128, D], BF16, tag="gath", bufs=6,
                                         name="gm")
                            nc.gpsimd.indirect_dma_start(
                                out=gm[:pb], out_offset=None, in_=emb_bf[:],
                                in_offset=bass.IndirectOffsetOnAxis(
                                    ap=vidx[:pb, 0:1], axis=0))
                            for kt in range(3):
                                w = 128 if kt < 2 else D - 256
                                tr(dec20[:w, kt, off:off + pb],
                                   gm[:pb, kt * 128:kt * 128 + w], pb, w)

    nc.compile()
    return nc


_NC_CACHE = None


def _get_nc():
    global _NC_CACHE
    if _NC_CACHE is None:
        _NC_CACHE = build_nc()
    return _NC_CACHE


def _pad_tiles(a, ntiles):
    rows, cols = a.shape
    out = np.zeros((128 * ntiles, cols), a.dtype)
    out[:rows] = a
    return np.ascontiguousarray(
        out.reshape(ntiles, 128, cols).transpose(1, 0, 2))


def _prep_shared(inputs):
    bf = np.float16
    f32 = np.float32
    eW = np.asarray(inputs["embed_W"], f32)
    d = {}
    wih = np.asarray(inputs["dec_W_ih"], f32)
    bih = np.asarray(inputs["dec_b_ih"], f32)
    bhh = np.asarray(inputs["dec_b_hh"], f32)
    gi = np.zeros((128 * 11, 3 * H), f32)
    gi[0:D] = wih[:, 0:D].T
    gi[320] = bih + np.concatenate([bhh[:2 * H], np.zeros(H, f32)])
    gi[384:384 + H] = wih[:, D:D + H].T
    gi[896:896 + H] = wih[:, D + H:].T
    d["w_gi"] = _pad_tiles(gi.astype(bf), 11)
    d["w_gh"] = _pad_tiles(np.asarray(inputs["dec_W_hh"], f32).T.astype(bf), 4)
    d["bhh_n"] = np.ascontiguousarray(bhh[2 * H:].astype(bf)[None, :])
    ewih = np.asarray(inputs["enc_W_ih"], f32)
    ebih = np.asarray(inputs["enc_b_ih"], f32)
    ebhh = np.asarray(inputs["enc_b_hh"], f32)
    egi = np.zeros((128 * 3, 3 * H), f32)
    egi[0:D] = ewih[:, :D].T
    egi[320] = ebih + np.concatenate([ebhh[:2 * H], np.zeros(H, f32)])
    d["w_egi"] = _pad_tiles(egi.astype(bf), 3)
    d["w_egh"] = _pad_tiles(np.asarray(inputs["enc_W_hh"], f32).T.astype(bf), 4)
    d["ebhh_n"] = np.ascontiguousarray(ebhh[2 * H:].astype(bf)[None, :])
    d["w_out"] = _pad_tiles(np.asarray(inputs["out_W"], f32).T.astype(bf), 12)
    d["outb"] = np.ascontiguousarray(
        np.asarray(inputs["out_b"], f32).astype(bf)[None, :])
    d["w_qk"] = _pad_tiles(np.asarray(inputs["qk_W"], f32).T.astype(bf), 4)
    d["w_qv"] = _pad_tiles(np.asarray(inputs["qv_W"], f32).T.astype(bf), 4)
    d["qvb_c"] = np.ascontiguousarray(
        np.asarray(inputs["qv_b"], f32).reshape(4, 128).T)
    d["w_ak"] = _pad_tiles(np.asarray(inputs["ak_W"], f32).T.astype(bf), 4)
    d["akb"] = np.ascontiguousarray(
        np.asarray(inputs["ak_b"], f32).astype(bf)[None, :])
    d["w_ik"] = _pad_tiles(np.asarray(inputs["ik_W"], f32).T.astype(bf), 2)
    d["w_iv"] = _pad_tiles(np.asarray(inputs["iv_W"], f32).T.astype(bf), 2)
    d["ivb_c"] = np.ascontiguousarray(
        np.asarray(inputs["iv_b"], f32).reshape(4, 128).T)
    d["emb_bf"] = eW.astype(bf)
    wd_b = np.asarray(inputs["wd_b"], f32)
    d["emb_aug"] = np.ascontiguousarray(
        np.concatenate([eW, wd_b[:, None]], 1))
    aug = np.zeros((128 * 3, VP), f32)
    aug[:D, :V] = eW.T
    aug[320, :V] = wd_b
    import ml_dtypes
    d["embt_bf"] = _pad_tiles(aug.astype(ml_dtypes.float8_e4m3fn), 3)
    return d


def _idx_cols(seq_rows):
    out = np.zeros((128, 2 * L), np.uint32)
    for t in range(L):
        out[:, 2 * t] = seq_rows[0:128, t]
        out[:32, 2 * t + 1] = seq_rows[128:160, t]
    return out


def _build_maps(inputs, shared):
    f32 = np.float32
    bf = np.float16
    ques = np.asarray(inputs["ques_seqs"]).astype(np.uint32)
    ans = np.asarray(inputs["ans_seqs"]).astype(np.uint32)
    qlens = np.asarray(inputs["ques_lens"]).astype(np.int64)
    img = np.asarray(inputs["img_seqs"], f32)
    maps = []
    for s in range(NCORES):
        m = dict(shared)
        r0 = s * BS
        m["q_idx"] = _idx_cols(ques[r0:r0 + BS, :L])
        m["a_idx"] = _idx_cols(ans[r0:r0 + BS, :L])
        qm = np.full((128, 2, L), NEG, f32)
        lens = qlens[r0:r0 + BS]
        for bt, (pb, off) in enumerate(zip(PBS, BOFF)):
            for b in range(pb):
                qm[b, bt, :lens[off + b]] = 0.0
        m["qe_mask"] = qm.astype(bf)
        im = np.full((128, 2, 256), NEG, f32)
        for bt, (pb, off) in enumerate(zip(PBS, BOFF)):
            for b in range(pb):
                gimg = (off + b) // ROUNDS
                im[b, bt, gimg * 16:(gimg + 1) * 16] = 0.0
        m["ie_mask"] = im.astype(bf)
        imgs = img[s * 16:(s + 1) * 16].reshape(256, 256)
        it = np.zeros((128 * 2, 256), f32)
        it[:256] = imgs.T
        m["img_t"] = np.ascontiguousarray(
            it.reshape(2, 128, 256).transpose(1, 0, 2)).astype(bf)
        maps.append(m)
    return maps


def kernel(**inputs):
    nc = _get_nc()
    shared = _prep_shared(inputs)
    in_maps = _build_maps(inputs, shared)
    from concourse.bass_utils import run_bass_kernel_spmd
    res = run_bass_kernel_spmd(nc, in_maps, core_ids=list(range(NCORES)))
    outs = []
    for s in range(NCORES):
        o = np.asarray(res.results[s]["out_o"])      # [21, 128, 3, 160]
        o19 = np.asarray(res.results[s]["out_o19"])  # [128, 2, 300]
        full = o.transpose(3, 0, 2, 1).reshape(BS, MAX_LEN, 384)[:, :, :D]
        full = np.ascontiguousarray(full)
        full[:128, MAX_LEN - 2] = o19[:, 0, :]
        full[128:, MAX_LEN - 2] = o19[:32, 1, :]
        outs.append(full)
    return np.concatenate(outs, 0).astype(np.float32)
